# revision 14
# baseline (speedup 1.0000x reference)
"""Trainium2 Bass kernel for nn_AttentionBlock (GroupNorm + 8-head attention
block on [8, 512, 32, 32], residual).

Sharding: pure data-parallel over batch B=8 across the 8 NeuronCores — one
batch element per core, weights replicated, zero collectives.

v5 = v4 (fp8e4 DoubleRow matmuls) + schedule/latency rework:
  - Head: x tiles stream in halves across both DMA queues before any other
    load; GN sums ride the idle ACT engine (Copy+accum), sum-of-squares on
    DVE, and the per-group algebra chain is DVE-resident (fewer cross-engine
    sem hops). Memsets issue after the loads.
  - Denominators: the den row is DMA'd straight out of the po PSUM (f32, no
    bf16 staging); reciprocals are scaled by 32 and broadcast per head as
    bf16 via the DRAM round trip (pairs 0-2) or a PE ones-matmul into a
    [128,1024] PSUM tile (pair 3, lowest latency).
  - attn: po rows are cast PSUM->fp8 with a 2^-5 prescale (unnormalized
    values reach ~733 > fp8 max), the odd head's rows hop partitions by DMA
    *before* the reciprocal arrives, and normalization happens in place
    (attn *= 32/den). This frees po banks at cast time and removes the au65
    staging tiles of v3/v4.
  - exp: odd-p sub1 tiles compute on the Pool engine via a Schraudolph
    bit-trick (uint8 = 1.4427*l + 32.46 IS the fp8e4m3 bit pattern of
    exp(l/8 - 2), ~3% rel err) so the ACT engine stops pacing the
    logits->exp->out2 pipeline. Attention carries ~7.6% of the output norm,
    so these approximations cost ~0.3% end-to-end (measured 6.3e-3 total,
    tolerance 2e-2).
  - proj_out: DoubleRow over chan-tile pairs, first-half accumulations run
    during the attention tail, output DMA'd as bf16.
"""
import sys

sys.path.insert(0, "/opt/trn_rl_repo")

import numpy as np
import ml_dtypes

import concourse.bass as bass
import concourse.bacc as bacc
import concourse.tile as tile
from concourse import mybir
from concourse.bass_utils import run_bass_kernel_spmd

F32 = mybir.dt.float32
BF16 = mybir.dt.bfloat16
FP8 = mybir.dt.float8e4
U8 = mybir.dt.uint8
ADD = mybir.AluOpType.add
MULT = mybir.AluOpType.mult
SUB = mybir.AluOpType.subtract
DR = mybir.MatmulPerfMode.DoubleRow

B, C, H, W = 8, 512, 32, 32
HW = H * W       # 1024
NG = 32          # groups
GS = C // NG     # 16 channels per group
NH = 8           # heads
HD = 64          # head dim
HID = NH * HD    # 512
NP = NH // 2     # 4 head pairs
EPS = 1e-6
SCALE = 1.0 / float(np.sqrt(HD))  # 0.125
EXP_SHIFT = -2.0  # exp(scale*l + shift): keeps e' under fp8e4 max (240)
ATT_PRE = 1.0 / 32  # prescale for the unnormalized po->fp8 cast
# Schraudolph fp8e4m3 bit-pattern exp: u8 = SCH_A*logit + SCH_B
SCH_A = 8.0 / np.log(2.0) * SCALE            # 1.44270
SCH_B = 8.0 * (np.log2(np.e) * EXP_SHIFT + 7.0) - 0.458
CT = C // 128    # 4 channel partition-tiles
PT = HW // 128   # 8 pixel partition-tiles
NA = PT // 2     # 4 kpix-tile pairs (DoubleRow accumulation steps)
GPT = NG // CT   # 8 groups per channel-tile
GN_INV = 1.0 / (GS * HW)          # 1/16384


def build_graph():
    nc = bacc.Bacc("TRN2", num_devices=8)

    x_ext = nc.declare_dram_parameter("x", [C, HW], BF16, isOutput=False)
    # fp8 pair-packed weights: [a][p, i, m] with contraction chan 128(2a+i)+p
    wqk_ext = nc.declare_dram_parameter("wqk8", [128, 2 * 2 * 1024], FP8,
                                        isOutput=False)
    wv_ext = nc.declare_dram_parameter("wv8", [128, 2 * 2 * 512], FP8,
                                       isOutput=False)
    wo_ext = nc.declare_dram_parameter("wo8", [128, 2 * 2 * 512], FP8,
                                       isOutput=False)
    # packed [128, 28] consts: 0:4 gamma, 4:8 beta, 8:16 b_in(q,k),
    # 16:20 b_out_eff, 20:28 gn_sel
    cpack_ext = nc.declare_dram_parameter("cpack", [128, 28], F32, isOutput=False)
    selT_ext = nc.declare_dram_parameter("gn_selT", [GPT + 2, 128], F32,
                                        isOutput=False)
    out_ext = nc.declare_dram_parameter("out", [C, HW], BF16, isOutput=True)

    rden_dram = nc.dram_tensor("rden_scratch", [NH, HW], BF16)

    with tile.TileContext(nc) as tc:
        with (
            tc.tile_pool(name="const", bufs=1) as const,
            tc.tile_pool(name="big", bufs=1) as big,
            tc.tile_pool(name="eT", bufs=1) as eTp,
            tc.tile_pool(name="small", bufs=2) as small,
        ):
            # ---------- loads: x0/x1 in halves across both queues first
            # (they gate the GN chain), then x2/x3, consts, weights ----------
            x_sb = [big.tile([128, HW], BF16, tag=f"x{t}", name=f"x{t}")
                    for t in range(CT)]
            for t in (0, 1):
                nc.gpsimd.dma_start(out=x_sb[t][:, 0:512],
                                    in_=x_ext[128 * t:128 * (t + 1), 0:512])
                nc.sync.dma_start(out=x_sb[t][:, 512:1024],
                                  in_=x_ext[128 * t:128 * (t + 1), 512:1024])
            nc.gpsimd.dma_start(out=x_sb[2], in_=x_ext[256:384, :])
            nc.sync.dma_start(out=x_sb[3], in_=x_ext[384:512, :])
            cpack_sb = const.tile([128, 28], F32)
            nc.gpsimd.dma_start(out=cpack_sb, in_=cpack_ext[:, :])
            selT_sb = const.tile([GPT, 128], F32)
            nc.gpsimd.dma_start(out=selT_sb, in_=selT_ext[0:GPT, :])
            gamma_sb = cpack_sb[:, 0:4]
            beta_sb = cpack_sb[:, 4:8]
            b_in_sb = cpack_sb[:, 8:16]
            b_out_sb = cpack_sb[:, 16:20]
            sel_sb = cpack_sb[:, 20:28]
            # fp8 weight pair-tiles
            wqk_sb = [big.tile([128, 2, 2 * HID], FP8, tag=f"wqk{a}",
                               name=f"wqk{a}") for a in range(2)]
            for a in range(2):
                nc.sync.dma_start(
                    out=wqk_sb[a][:, :, :],
                    in_=wqk_ext[:, 2 * HID * 2 * a:2 * HID * 2 * (a + 1)]
                    .rearrange("p (i m) -> p i m", i=2))
            wv_sb = [big.tile([128, 2, HID], FP8, tag=f"wv{a}",
                              name=f"wv{a}") for a in range(2)]
            for a in range(2):
                nc.sync.dma_start(
                    out=wv_sb[a][:, :, :],
                    in_=wv_ext[:, HID * 2 * a:HID * 2 * (a + 1)]
                    .rearrange("p (i m) -> p i m", i=2))
            wo_sb = [big.tile([128, 2, HID], FP8, tag=f"wo{a}",
                              name=f"wo{a}") for a in range(2)]
            for a in range(2):
                nc.sync.dma_start(
                    out=wo_sb[a][:, :, :],
                    in_=wo_ext[:, HID * 2 * a:HID * 2 * (a + 1)]
                    .rearrange("p (i m) -> p i m", i=2))
            eshift_sb = const.tile([128, 1], F32)
            nc.vector.memset(eshift_sb, float(EXP_SHIFT))
            one_sb = const.tile([128, 1], F32)
            nc.vector.memset(one_sb, 1.0)
            # dummy ops hoist the ACT table loads (Sqrt/Identity and Exp
            # sets) into the idle pre-x window instead of the GN/exp path
            tl_scratch = small.tile([128, 1], F32, tag="tls", bufs=1)
            nc.scalar.activation(out=tl_scratch, in_=one_sb,
                                 func=mybir.ActivationFunctionType.Sqrt,
                                 bias=one_sb[:, :], scale=1.0)
            nc.scalar.activation(out=tl_scratch, in_=one_sb,
                                 func=mybir.ActivationFunctionType.Exp,
                                 scale=1.0, bias=one_sb[:, :])

            # ---------- SBUF state ----------
            # h in fp8 pair-tiles: h_pair[a][:, i, :] = GN output chan-tile 2a+i
            h_pair = [big.tile([128, 2, HW], FP8, tag=f"h{a}", name=f"h{a}")
                      for a in range(2)]
            q_sb = [big.tile([128, HW], BF16, tag=f"q{m}", name=f"q{m}")
                    for m in range(NP)]
            k_sb = [big.tile([128, HW], BF16, tag=f"k{m}", name=f"k{m}")
                    for m in range(NP)]
            # vT pair-tiles: [a][p, i, head, c] = v for kpix 128(2a+i)+p,
            # c==HD is the denominator ones column; head-dim padded to HD+2
            # so the DoubleRow pair-stride stays 16B-aligned
            vT_pair = [big.tile([128, 2, NH, HD + 2], FP8, tag=f"vT{a}",
                                name=f"vT{a}") for a in range(NA)]
            # attn pair-tiles: [g][p, i, n] = attn chans 128(2g+i)+p
            attn_pair = [big.tile([128, 2, HW], FP8, tag=f"at{g}",
                                  name=f"at{g}") for g in range(2)]
            po_tiles = {}    # hp -> [po_sub0, po_sub1]
            eT_all = {}      # hp -> [[eT pair tiles sub0], [sub1]]

            def qk_burst(hp, which):
                """DoubleRow q or k projection for pair hp (4 matmuls)."""
                dest, off, bc = ((q_sb, 0, hp) if which == "q"
                                 else (k_sb, HID, 4 + hp))
                pp = pbig.tile([128, HW], F32, tag="pb",
                               name=f"{which}_acc{hp}")
                for n in range(2):
                    for a in range(2):
                        nc.tensor.matmul(
                            pp[:, 512 * n:512 * (n + 1)],
                            lhsT=wqk_sb[a][:, :, off + 128 * hp:
                                           off + 128 * (hp + 1)],
                            rhs=h_pair[a][:, :, 512 * n:512 * (n + 1)],
                            start=(a == 0), stop=(a == 1), perf_mode=DR)
                nc.vector.tensor_scalar(
                    out=dest[hp], in0=pp[:, :],
                    scalar1=b_in_sb[:, bc:bc + 1], scalar2=None, op0=ADD)

            def v_one(p):
                """One v kpix-tile (pv pool, own PSUM banks), DoubleRow."""
                pp = pv.tile([128, 512], F32, tag="pv")
                for a in range(2):
                    nc.tensor.matmul(
                        pp[:, :],
                        lhsT=h_pair[a][:, :, 128 * p:128 * (p + 1)],
                        rhs=wv_sb[a][:, :, :],
                        start=(a == 0), stop=(a == 1), perf_mode=DR)
                nc.vector.tensor_copy(
                    out=vT_pair[p // 2][:, p % 2, :, 0:HD],
                    in_=pp[:, :].rearrange("a (nh c) -> a nh c", nh=NH))

            def out2_step(hp, a, subs=(0, 1)):
                """One DoubleRow kpix-pair accumulation of pair hp's out2."""
                if a == 0 and hp not in po_tiles:
                    po_tiles[hp] = [
                        pop.tile([HD + 1, HW], F32, tag="po",
                                 name=f"po{2 * hp + s}") for s in range(2)]
                eTs = eT_all[hp]
                for sub in subs:
                    head = 2 * hp + sub
                    po_t = po_tiles[hp][sub]
                    for n in range(2):
                        nc.tensor.matmul(
                            po_t[:, 512 * n:512 * (n + 1)],
                            lhsT=vT_pair[a][:, :, head, 0:HD + 1],
                            rhs=eTs[sub][a][:, :, 512 * n:512 * (n + 1)],
                            start=(a == 0), stop=(a == NA - 1), perf_mode=DR)

            def emit_logits_exp(hp, out2_of=None, fillers=None,
                                o2_defer=()):
                """Logits (bf16) + exp->fp8 for pair hp; out2 DR steps of
                pair out2_of ride at odd p (when an eT pair completes).
                sub1 exps at odd p run on Pool via the Schraudolph trick."""
                eTs = [[eTp.tile([128, 2, HW], FP8, bufs=2, tag=f"eT{sub}_{a}",
                                 name=f"eT{hp}_{sub}_{a}") for a in range(NA)]
                       for sub in range(2)]
                eT_all[hp] = eTs
                o2_queue = []
                for p in range(PT):
                    pls = []
                    for sub in range(2):
                        lo = 64 * sub
                        pl = pbig.tile([128, HW], F32, tag="pb",
                                       name=f"pl{hp}_{sub}_{p}")
                        for n in range(2):
                            nc.tensor.matmul(
                                pl[:, 512 * n:512 * (n + 1)],
                                lhsT=k_sb[hp][lo:lo + 64, 128 * p:128 * (p + 1)],
                                rhs=q_sb[hp][lo:lo + 64, 512 * n:512 * (n + 1)],
                                start=True, stop=True)
                        pls.append(pl)
                    for sub in range(2):
                        dst = eTs[sub][p // 2][:, p % 2, :]
                        if sub == 1 and p % 2 == 1 and hp == 3:
                            # Schraudolph bit-trick exp on DVE: relieves the
                            # ACT engine in the out2(3) chase region
                            nc.vector.tensor_scalar(
                                out=dst.bitcast(U8), in0=pls[sub][:, :],
                                scalar1=float(SCH_A), scalar2=float(SCH_B),
                                op0=MULT, op1=ADD)
                        else:
                            nc.scalar.activation(
                                out=dst, in_=pls[sub][:, :],
                                func=mybir.ActivationFunctionType.Exp,
                                scale=SCALE, bias=eshift_sb[:, :])
                    if out2_of is not None and p % 2 == 1:
                        a = p // 2
                        if a in o2_defer:
                            o2_queue.append(a)
                        else:
                            for qq in o2_queue:
                                out2_step(out2_of, qq)
                            o2_queue.clear()
                            out2_step(out2_of, a)
                    for f in (fillers or {}).get(p, []):
                        f()

            pair_state = {}

            def finish_den(hp, pe_bcast=False):
                """po -> attn (fp8, prescaled 2^-5, unnormalized) + den
                reciprocal. The odd head's partition hop rides a DMA that
                doesn't wait for the reciprocal; normalization is done in
                place afterwards by finish_mul."""
                eT_all.pop(hp)
                pos = po_tiles.pop(hp)
                g, i = hp // 2, hp % 2
                den8 = small.tile([2, HW], FP8, tag="den8", bufs=2,
                                  name=f"den8_{hp}")
                den2 = small.tile([2, HW], F32, tag="den2", bufs=2,
                                  name=f"den2_{hp}")
                rr2 = small.tile([2, HW], F32, tag="rr2", bufs=2,
                                 name=f"rr2_{hp}")
                den_eng = nc.gpsimd if pe_bcast else nc.sync
                hop_eng = nc.sync if pe_bcast else nc.gpsimd
                # one prescaled fp8 cast per head covers attn rows AND the
                # denominator row; the 1/32 prescale cancels exactly in
                # attn = au8 * (1/(den/32))
                au8s = []
                for sub in range(2):
                    au8 = small.tile([HD + 1, HW], FP8, tag="attnu",
                                     bufs=4, name=f"attnu{2 * hp + sub}")
                    nc.vector.tensor_scalar_mul(au8, pos[sub][:, :],
                                                float(ATT_PRE))
                    den_eng.dma_start(out=den8[sub:sub + 1, :],
                                      in_=au8[HD:HD + 1, :])
                    au8s.append(au8)
                # odd head's partition hop: unnormalized, doesn't wait for
                # rb; separate queue so it overlaps the den extracts
                hop_eng.dma_start(out=attn_pair[g][HD:128, i, :],
                                  in_=au8s[1][0:HD, :])
                # fp8->f32 widen on ACT (Identity + ~0 bias): parallel with
                # the DVE casts; then the DVE reciprocal
                nc.scalar.activation(
                    out=den2, in_=den8,
                    func=mybir.ActivationFunctionType.Identity,
                    bias=eps_sb[0:2, :], scale=1.0)
                nc.vector.reciprocal_approx_fast(out=rr2, in_=den2)
                if pe_bcast:
                    # one matmul pair broadcasts BOTH heads: indicator rows
                    # of selT pick rr2 row 0 for partitions 0-63, row 1 for
                    # 64-127
                    bb = pop.tile([128, HW], F32, tag="po", name=f"bb{hp}")
                    for n in range(2):
                        nc.tensor.matmul(
                            bb[:, 512 * n:512 * (n + 1)],
                            lhsT=ind2_sb[:, :],
                            rhs=rr2[0:2, 512 * n:512 * (n + 1)],
                            start=True, stop=True)
                    pair_state[hp] = (au8s[0], bb)
                else:
                    rr2b = small.tile([2, HW], BF16, tag="rr2b", bufs=2,
                                      name=f"rr2b_{hp}")
                    nc.vector.tensor_copy(out=rr2b, in_=rr2)
                    for sub in range(2):
                        nc.sync.dma_start(
                            out=rden_dram[2 * hp + sub:2 * hp + sub + 1, :],
                            in_=rr2b[sub:sub + 1, :])
                    pair_state[hp] = (au8s[0], None)

            def finish_mul(hp):
                """Normalize: even head au8 x rb -> attn; odd head (already
                hopped, unnormalized) in place."""
                au8e, bb = pair_state.pop(hp)
                g, i = hp // 2, hp % 2
                if bb is None:
                    rbt = small.tile([128, HW], BF16, tag="rb", bufs=2,
                                     name=f"rb{hp}")
                    for sub in range(2):
                        bcast_ap = bass.AP(
                            tensor=rden_dram[:, :].tensor,
                            offset=(2 * hp + sub) * HW,
                            ap=[[0, HD], [1, HW]])
                        nc.sync.dma_start(out=rbt[64 * sub:64 * (sub + 1), :],
                                          in_=bcast_ap)
                    rbs = [rbt[0:HD, :], rbt[HD:128, :]]
                else:
                    rbs = [bb[0:HD, :], bb[HD:128, :]]
                eng = nc.vector if bb is not None else nc.gpsimd
                eng.tensor_mul(attn_pair[g][0:HD, i, :],
                               au8e[0:HD, :], rbs[0])
                eng.tensor_mul(attn_pair[g][HD:128, i, :],
                               attn_pair[g][HD:128, i, :], rbs[1])

            with tc.tile_pool(name="pbig", bufs=2, space="PSUM") as pbig:
                # GN per tile + qk0 DR accumulation rides along so the first
                # exp fires as soon as x1 lands.
                ppq0 = pbig.tile([128, HW], F32, tag="pb", name="q_acc0")
                ppk0 = pbig.tile([128, HW], F32, tag="pb", name="k_acc0")
                with tc.tile_pool(name="ps_gn", bufs=2, space="PSUM") as ps_gn:
                    eps_sb = small.tile([GPT, 1], F32, tag="eps_c", bufs=1)
                    nc.gpsimd.memset(eps_sb, float(EPS))
                    sq_scratch = small.tile([128, HW], BF16, tag="sqs",
                                            bufs=1)
                    sts = []

                    def gn_stats(t):
                        st = small.tile([128, 2], F32, tag=f"st{t}", bufs=1,
                                        name=f"st{t}")
                        sts.append(st)
                        nc.vector.reduce_sum(st[:, 0:1], x_sb[t][:, :],
                                             axis=mybir.AxisListType.X)
                        nc.vector.scalar_tensor_tensor(
                            out=sq_scratch, in0=x_sb[t][:, :], scalar=1.0,
                            in1=x_sb[t][:, :],
                            op0=mybir.AluOpType.bypass, op1=MULT,
                            accum_out=st[:, 1:2])

                    def gn_tile(t):
                        st = sts[t]
                        gpsum = ps_gn.tile([GPT, 2], F32, tag="gps")
                        nc.tensor.matmul(gpsum[:, :], lhsT=sel_sb[:, :],
                                         rhs=st[:, :], start=True, stop=True)
                        # grp cols: 0 rstd, 1 mean*rstd, 2 mean, 3 E[x^2]
                        grp = small.tile([GPT, 4], F32, tag="grp", bufs=2,
                                         name=f"grp{t}")
                        nc.vector.tensor_scalar_mul(grp[:, 2:4],
                                                    gpsum[:, 0:2], GN_INV)
                        nc.vector.tensor_mul(grp[:, 0:1], grp[:, 2:3],
                                             grp[:, 2:3])
                        nc.vector.tensor_sub(grp[:, 0:1], grp[:, 3:4],
                                             grp[:, 0:1])
                        nc.scalar.activation(
                            out=grp[:, 0:1], in_=grp[:, 0:1],
                            func=mybir.ActivationFunctionType.Sqrt,
                            bias=eps_sb[:, :], scale=1.0)
                        nc.vector.reciprocal(out=grp[:, 0:1], in_=grp[:, 0:1])
                        nc.vector.tensor_mul(grp[:, 1:2], grp[:, 2:3],
                                             grp[:, 0:1])
                        epsum = ps_gn.tile([128, 2], F32, tag="eps")
                        nc.tensor.matmul(epsum[:, :], lhsT=selT_sb[:, :],
                                         rhs=grp[:, 0:2], start=True,
                                         stop=True)
                        ga = small.tile([128, 1], F32, tag=f"ga{t}", bufs=1,
                                        name=f"ga{t}")
                        gd = small.tile([128, 1], F32, tag=f"gd{t}", bufs=1,
                                        name=f"gd{t}")
                        nc.vector.tensor_mul(ga[:, :], gamma_sb[:, t:t + 1],
                                             epsum[:, 0:1])
                        nc.vector.tensor_mul(gd[:, :], gamma_sb[:, t:t + 1],
                                             epsum[:, 1:2])
                        nc.vector.tensor_sub(gd[:, :], beta_sb[:, t:t + 1],
                                             gd[:, :])
                        nc.vector.tensor_scalar(
                            out=h_pair[t // 2][:, t % 2, :],
                            in0=x_sb[t][:, :],
                            scalar1=ga[:, :], scalar2=gd[:, :],
                            op0=MULT, op1=ADD)

                    # stats for t0/t1 first, then each tile's chain as
                    # soon as its stats are in — keeps grp(t) from queueing
                    # behind later tiles' big DVE reductions
                    gn_stats(0)
                    gn_stats(1)
                    for t in range(CT):
                        if t == 2:
                            gn_stats(2)
                        if t == 3:
                            gn_stats(3)
                        gn_tile(t)
                        if t % 2 == 1:
                            a = t // 2
                            for n in range(2):
                                nc.tensor.matmul(
                                    ppq0[:, 512 * n:512 * (n + 1)],
                                    lhsT=wqk_sb[a][:, :, 0:128],
                                    rhs=h_pair[a][:, :, 512 * n:512 * (n + 1)],
                                    start=(a == 0), stop=(a == 1),
                                    perf_mode=DR)
                                nc.tensor.matmul(
                                    ppk0[:, 512 * n:512 * (n + 1)],
                                    lhsT=wqk_sb[a][:, :, HID:HID + 128],
                                    rhs=h_pair[a][:, :, 512 * n:512 * (n + 1)],
                                    start=(a == 0), stop=(a == 1),
                                    perf_mode=DR)
                nc.vector.tensor_scalar(
                    out=q_sb[0], in0=ppq0[:, :],
                    scalar1=b_in_sb[:, 0:1], scalar2=None, op0=ADD)
                # k0 evict on ACT (idle pre-exp) so it runs parallel to the
                # q0 evict on DVE — both gate the first logits matmul.
                nc.scalar.activation(
                    out=k_sb[0], in_=ppk0[:, :],
                    func=mybir.ActivationFunctionType.Identity,
                    bias=b_in_sb[:, 4:5], scale=1.0)
                for a in range(NA):
                    for i in range(2):
                        nc.vector.memset(vT_pair[a][:, i, :, HD:HD + 1], 1.0)
                ind2_sb = const.tile([2, 128], F32)
                nc.sync.dma_start(out=ind2_sb,
                                  in_=selT_ext[GPT:GPT + 2, :])
                with tc.tile_pool(name="pv", bufs=2, space="PSUM") as pv:
                    emit_logits_exp(0, fillers={
                        0: [lambda: v_one(0)],
                        1: [lambda: v_one(1)],
                        2: [lambda: qk_burst(1, "q")],
                        3: [lambda: v_one(2)],
                        4: [lambda: v_one(3)],
                        5: [lambda: qk_burst(1, "k")],
                        6: [lambda: v_one(4), lambda: v_one(5)],
                        7: [lambda: v_one(6), lambda: v_one(7)],
                    })
                with tc.tile_pool(name="po", bufs=2, space="PSUM") as pop:
                    emit_logits_exp(1, out2_of=0, o2_defer=(1,), fillers={
                        2: [lambda: qk_burst(2, "q")],
                        5: [lambda: qk_burst(2, "k")],
                    })
                    finish_den(0)
                    emit_logits_exp(2, out2_of=1, o2_defer=(1,), fillers={
                        2: [lambda: qk_burst(3, "q")],
                        5: [lambda: qk_burst(3, "k")],
                    })
                    finish_den(1)
                    finish_mul(0)
                    emit_logits_exp(3, out2_of=2)
                    finish_den(2)
                    finish_mul(1)
                    # chase pair 3 sub-major: sub0's po finishes (and its
                    # au cast + den extract start) while sub1 still matmuls
                    for a in range(NA):
                        out2_step(3, a, subs=(0,))
                    for a in range(NA):
                        out2_step(3, a, subs=(1,))
                    finish_den(3, pe_bcast=True)
                    finish_mul(2)
                    finish_mul(3)

            # ---------- proj_out + bias + residual ----------
            # DoubleRow over chan-tile pairs: 2 accumulation steps per chunk.
            # pre = step a=0 (runs as soon as attn_pair[0] lands), fin = a=1
            # + evict + bf16 DMA out.
            with tc.tile_pool(name="ps_pout", bufs=2, space="PSUM") as ps_pout:
                def pout_chunk_pre(m, pps):
                    pp = ps_pout.tile([128, HW], F32, tag="pp",
                                      name=f"po_{m}")
                    pps[m] = pp
                    for n in range(2):
                        nc.tensor.matmul(
                            pp[:, 512 * n:512 * (n + 1)],
                            lhsT=wo_sb[0][:, :, 128 * m:128 * (m + 1)],
                            rhs=attn_pair[0][:, :, 512 * n:512 * (n + 1)],
                            start=True, stop=False, perf_mode=DR)

                def pout_chunk_fin(m, pps):
                    pp = pps.pop(m)
                    for n in range(2):
                        nc.tensor.matmul(
                            pp[:, 512 * n:512 * (n + 1)],
                            lhsT=wo_sb[1][:, :, 128 * m:128 * (m + 1)],
                            rhs=attn_pair[1][:, :, 512 * n:512 * (n + 1)],
                            start=False, stop=True, perf_mode=DR)
                    o_sb = small.tile([128, HW], BF16, tag="osb", bufs=2)
                    nc.vector.scalar_tensor_tensor(
                        out=o_sb, in0=pp[:, :],
                        scalar=b_out_sb[:, m:m + 1],
                        in1=x_sb[m][:, :],
                        op0=ADD, op1=ADD)
                    for hh in range(4):
                        deng = nc.gpsimd if (m + hh) % 2 else nc.sync
                        deng.dma_start(
                            out=out_ext[128 * m:128 * (m + 1),
                                        256 * hh:256 * (hh + 1)],
                            in_=o_sb[:, 256 * hh:256 * (hh + 1)])

                pps = {}
                for m in (0, 1):
                    pout_chunk_pre(m, pps)
                for m in (0, 1):
                    pout_chunk_fin(m, pps)
                for m in (2, 3):
                    pout_chunk_pre(m, pps)
                for m in (2, 3):
                    pout_chunk_fin(m, pps)
    return nc


def _install_ntff_hook():
    """The agent image's antenv lacks axon_hooks; synthesize it so
    run_bass_kernel_spmd(trace=True) can reach the NTFF profiler."""
    import types
    if "antenv.axon_hooks" in sys.modules:
        return
    mod = types.ModuleType("antenv.axon_hooks")
    mod._hook = None

    def set_axon_ntff_profile_hook(hook):
        mod._hook = hook

    def get_axon_ntff_profile_hook():
        return mod._hook

    mod.set_axon_ntff_profile_hook = set_axon_ntff_profile_hook
    mod.get_axon_ntff_profile_hook = get_axon_ntff_profile_hook
    sys.modules["antenv.axon_hooks"] = mod
    try:
        from trn_agent_boot.trn_boot import _ntff_profile_via_ctypes
        hook = _ntff_profile_via_ctypes("/opt/axon/libaxon_pjrt.so")
        if hook is not None:
            set_axon_ntff_profile_hook(hook)
    except Exception as e:  # degrade to no tracing
        print("ntff hook setup failed:", e)


_COMPILED = None


def _get_compiled():
    global _COMPILED
    if _COMPILED is None:
        nc = build_graph()
        nc.compile()
        _COMPILED = nc
    return _COMPILED


def _make_consts():
    # within any 128-channel tile, partition p belongs to local group p//16;
    # rows GPT/GPT+1 are even/odd-head indicator rows for the den broadcast
    sel = np.zeros((128, GPT), dtype=np.float32)
    selT = np.zeros((GPT + 2, 128), dtype=np.float32)
    for p in range(128):
        sel[p, p // GS] = 1.0
        selT[p // GS, p] = 1.0
    selT[GPT, 0:64] = 1.0
    selT[GPT + 1, 64:128] = 1.0
    return sel, selT


def _pm(v, cols):
    """[cols*128] vector -> partition-major [128, cols]."""
    return np.ascontiguousarray(v.reshape(cols, 128).T)


def _pack_pairs(w, m_dim):
    """w [m_dim, 512] -> [128, 2*2*m_dim] fp8: flat[a] [p, i, m] =
    w[m, 128*(2a+i)+p]."""
    wT = w.T.reshape(2, 2, 128, m_dim)           # [a, i, p, m]
    out = np.transpose(wT, (0, 2, 1, 3))          # [a, p, i, m]
    out = out.reshape(2, 128, 2 * m_dim)          # [a, p, i*m]
    out = np.concatenate([out[0], out[1]], axis=1)  # [p, a*i*m]
    return np.ascontiguousarray(out).astype(ml_dtypes.float8_e4m3)


def kernel(x, gamma, beta, w_in, b_in, w_out, b_out, _trace=False):
    x = np.asarray(x, dtype=np.float32)
    gamma = np.asarray(gamma, dtype=np.float32)
    beta = np.asarray(beta, dtype=np.float32)
    w_in = np.asarray(w_in, dtype=np.float32)
    b_in = np.asarray(b_in, dtype=np.float32)
    w_out = np.asarray(w_out, dtype=np.float32)
    b_out = np.asarray(b_out, dtype=np.float32)

    wqk8 = _pack_pairs(w_in[0:2 * HID], 2 * HID)   # q rows 0:512, k 512:1024
    wv8 = _pack_pairs(w_in[2 * HID:3 * HID], HID)
    wo8 = _pack_pairs(w_out, HID)
    sel, selT = _make_consts()
    # fold v-bias through proj_out: softmax rows sum to 1, so the attention
    # output is attn_raw + b_v exactly; w_out @ b_v + b_out replaces b_out.
    b_v = b_in[2 * HID:3 * HID]
    b_out_eff = b_out + w_out.astype(np.float64) @ b_v.astype(np.float64)
    b_out_eff = b_out_eff.astype(np.float32)
    cpack = np.zeros((128, 28), dtype=np.float32)
    cpack[:, 0:4] = _pm(gamma, CT)
    cpack[:, 4:8] = _pm(beta, CT)
    cpack[:, 8:16] = _pm(b_in[0:2 * HID], 8)
    cpack[:, 16:20] = _pm(b_out_eff, CT)
    cpack[:, 20:28] = sel
    common = {
        "wqk8": wqk8,
        "wv8": wv8,
        "wo8": wo8,
        "cpack": cpack,
        "gn_selT": selT,
    }
    in_maps = []
    for b in range(B):
        m = dict(common)
        m["x"] = np.ascontiguousarray(x[b].reshape(C, HW)).astype(
            ml_dtypes.bfloat16)
        in_maps.append(m)

    if _trace:
        _install_ntff_hook()
    nc = _get_compiled()
    res = run_bass_kernel_spmd(nc, in_maps, core_ids=list(range(B)),
                               trace=_trace)
    out = np.stack([np.asarray(res.results[b]["out"]).astype(np.float32)
                    .reshape(C, H, W) for b in range(B)])
    if _trace:
        return out, res
    return out


if __name__ == "__main__":
    rng = np.random.default_rng(0)
    inputs = {
        "x": rng.standard_normal((B, C, H, W), dtype=np.float32),
        "gamma": np.ones(C, dtype=np.float32),
        "beta": np.zeros(C, dtype=np.float32),
        "w_in": (rng.standard_normal((3 * HID, C), dtype=np.float32)
                 / np.sqrt(C)),
        "b_in": np.zeros(3 * HID, dtype=np.float32),
        "w_out": (rng.standard_normal((C, HID), dtype=np.float32)
                  / np.sqrt(HID)),
        "b_out": np.zeros(C, dtype=np.float32),
    }
    out = kernel(**inputs)
    print("kernel ran, out shape", out.shape)


# revision 15
# speedup vs baseline: 1.0122x; 1.0122x over previous
"""Trainium2 Bass kernel for nn_AttentionBlock (GroupNorm + 8-head attention
block on [8, 512, 32, 32], residual).

Sharding: pure data-parallel over batch B=8 across the 8 NeuronCores — one
batch element per core, weights replicated, zero collectives.

v5 = v4 (fp8e4 DoubleRow matmuls) + schedule/latency rework:
  - Head: x tiles stream in halves across both DMA queues before any other
    load; GN sums ride the idle ACT engine (Copy+accum), sum-of-squares on
    DVE, and the per-group algebra chain is DVE-resident (fewer cross-engine
    sem hops). Memsets issue after the loads.
  - Denominators: the den row is DMA'd straight out of the po PSUM (f32, no
    bf16 staging); reciprocals are scaled by 32 and broadcast per head as
    bf16 via the DRAM round trip (pairs 0-2) or a PE ones-matmul into a
    [128,1024] PSUM tile (pair 3, lowest latency).
  - attn: po rows are cast PSUM->fp8 with a 2^-5 prescale (unnormalized
    values reach ~733 > fp8 max), the odd head's rows hop partitions by DMA
    *before* the reciprocal arrives, and normalization happens in place
    (attn *= 32/den). This frees po banks at cast time and removes the au65
    staging tiles of v3/v4.
  - exp: odd-p sub1 tiles compute on the Pool engine via a Schraudolph
    bit-trick (uint8 = 1.4427*l + 32.46 IS the fp8e4m3 bit pattern of
    exp(l/8 - 2), ~3% rel err) so the ACT engine stops pacing the
    logits->exp->out2 pipeline. Attention carries ~7.6% of the output norm,
    so these approximations cost ~0.3% end-to-end (measured 6.3e-3 total,
    tolerance 2e-2).
  - proj_out: DoubleRow over chan-tile pairs, first-half accumulations run
    during the attention tail, output DMA'd as bf16.
"""
import sys

sys.path.insert(0, "/opt/trn_rl_repo")

import numpy as np
import ml_dtypes

import concourse.bass as bass
import concourse.bacc as bacc
import concourse.tile as tile
from concourse import mybir
from concourse.bass_utils import run_bass_kernel_spmd

F32 = mybir.dt.float32
BF16 = mybir.dt.bfloat16
FP8 = mybir.dt.float8e4
U8 = mybir.dt.uint8
ADD = mybir.AluOpType.add
MULT = mybir.AluOpType.mult
SUB = mybir.AluOpType.subtract
DR = mybir.MatmulPerfMode.DoubleRow

B, C, H, W = 8, 512, 32, 32
HW = H * W       # 1024
NG = 32          # groups
GS = C // NG     # 16 channels per group
NH = 8           # heads
HD = 64          # head dim
HID = NH * HD    # 512
NP = NH // 2     # 4 head pairs
EPS = 1e-6
SCALE = 1.0 / float(np.sqrt(HD))  # 0.125
EXP_SHIFT = -2.0  # exp(scale*l + shift): keeps e' under fp8e4 max (240)
ATT_PRE = 1.0 / 32  # prescale for the unnormalized po->fp8 cast
# Schraudolph fp8e4m3 bit-pattern exp: u8 = SCH_A*logit + SCH_B
SCH_A = 8.0 / np.log(2.0) * SCALE            # 1.44270
SCH_B = 8.0 * (np.log2(np.e) * EXP_SHIFT + 7.0) - 0.458
CT = C // 128    # 4 channel partition-tiles
PT = HW // 128   # 8 pixel partition-tiles
NA = PT // 2     # 4 kpix-tile pairs (DoubleRow accumulation steps)
GPT = NG // CT   # 8 groups per channel-tile
GN_INV = 1.0 / (GS * HW)          # 1/16384


def build_graph():
    nc = bacc.Bacc("TRN2", num_devices=8)

    x_ext = nc.declare_dram_parameter("x", [C, HW], BF16, isOutput=False)
    # fp8 pair-packed weights: [a][p, i, m] with contraction chan 128(2a+i)+p
    wqk_ext = nc.declare_dram_parameter("wqk8", [128, 2 * 2 * 1024], FP8,
                                        isOutput=False)
    wv_ext = nc.declare_dram_parameter("wv8", [128, 2 * 2 * 512], FP8,
                                       isOutput=False)
    wo_ext = nc.declare_dram_parameter("wo8", [128, 2 * 2 * 512], FP8,
                                       isOutput=False)
    # packed [128, 28] consts: 0:4 gamma, 4:8 beta, 8:16 b_in(q,k),
    # 16:20 b_out_eff, 20:28 gn_sel
    cpack_ext = nc.declare_dram_parameter("cpack", [128, 28], F32, isOutput=False)
    selT_ext = nc.declare_dram_parameter("gn_selT", [GPT + 2, 128], F32,
                                        isOutput=False)
    out_ext = nc.declare_dram_parameter("out", [C, HW], BF16, isOutput=True)

    rden_dram = nc.dram_tensor("rden_scratch", [NH, HW], BF16)

    with tile.TileContext(nc) as tc:
        with (
            tc.tile_pool(name="const", bufs=1) as const,
            tc.tile_pool(name="big", bufs=1) as big,
            tc.tile_pool(name="eT", bufs=1) as eTp,
            tc.tile_pool(name="small", bufs=2) as small,
        ):
            # ---------- loads: x0/x1 in halves across both queues first
            # (they gate the GN chain), then x2/x3, consts, weights ----------
            x_sb = [big.tile([128, HW], BF16, tag=f"x{t}", name=f"x{t}")
                    for t in range(CT)]
            for t in (0, 1):
                nc.gpsimd.dma_start(out=x_sb[t][:, 0:512],
                                    in_=x_ext[128 * t:128 * (t + 1), 0:512])
                nc.sync.dma_start(out=x_sb[t][:, 512:1024],
                                  in_=x_ext[128 * t:128 * (t + 1), 512:1024])
            nc.gpsimd.dma_start(out=x_sb[2], in_=x_ext[256:384, :])
            nc.sync.dma_start(out=x_sb[3], in_=x_ext[384:512, :])
            cpack_sb = const.tile([128, 28], F32)
            nc.gpsimd.dma_start(out=cpack_sb, in_=cpack_ext[:, :])
            selT_sb = const.tile([GPT, 128], F32)
            nc.gpsimd.dma_start(out=selT_sb, in_=selT_ext[0:GPT, :])
            gamma_sb = cpack_sb[:, 0:4]
            beta_sb = cpack_sb[:, 4:8]
            b_in_sb = cpack_sb[:, 8:16]
            b_out_sb = cpack_sb[:, 16:20]
            sel_sb = cpack_sb[:, 20:28]
            # fp8 weight pair-tiles
            wqk_sb = [big.tile([128, 2, 2 * HID], FP8, tag=f"wqk{a}",
                               name=f"wqk{a}") for a in range(2)]
            for a in range(2):
                nc.sync.dma_start(
                    out=wqk_sb[a][:, :, :],
                    in_=wqk_ext[:, 2 * HID * 2 * a:2 * HID * 2 * (a + 1)]
                    .rearrange("p (i m) -> p i m", i=2))
            wv_sb = [big.tile([128, 2, HID], FP8, tag=f"wv{a}",
                              name=f"wv{a}") for a in range(2)]
            for a in range(2):
                nc.sync.dma_start(
                    out=wv_sb[a][:, :, :],
                    in_=wv_ext[:, HID * 2 * a:HID * 2 * (a + 1)]
                    .rearrange("p (i m) -> p i m", i=2))
            wo_sb = [big.tile([128, 2, HID], FP8, tag=f"wo{a}",
                              name=f"wo{a}") for a in range(2)]
            for a in range(2):
                nc.sync.dma_start(
                    out=wo_sb[a][:, :, :],
                    in_=wo_ext[:, HID * 2 * a:HID * 2 * (a + 1)]
                    .rearrange("p (i m) -> p i m", i=2))
            eshift_sb = const.tile([128, 1], F32)
            nc.vector.memset(eshift_sb, float(EXP_SHIFT))
            one_sb = const.tile([128, 1], F32)
            nc.vector.memset(one_sb, 1.0)
            # dummy ops hoist the ACT table loads (Sqrt/Identity and Exp
            # sets) into the idle pre-x window instead of the GN/exp path
            tl_scratch = small.tile([128, 1], F32, tag="tls", bufs=1)
            nc.scalar.activation(out=tl_scratch, in_=one_sb,
                                 func=mybir.ActivationFunctionType.Sqrt,
                                 bias=one_sb[:, :], scale=1.0)
            nc.scalar.activation(out=tl_scratch, in_=one_sb,
                                 func=mybir.ActivationFunctionType.Exp,
                                 scale=1.0, bias=one_sb[:, :])
            nc.scalar.activation(out=tl_scratch, in_=one_sb,
                                 func=mybir.ActivationFunctionType.Copy)

            # ---------- SBUF state ----------
            # h in fp8 pair-tiles: h_pair[a][:, i, :] = GN output chan-tile 2a+i
            h_pair = [big.tile([128, 2, HW], FP8, tag=f"h{a}", name=f"h{a}")
                      for a in range(2)]
            q_sb = [big.tile([128, HW], BF16, tag=f"q{m}", name=f"q{m}")
                    for m in range(NP)]
            k_sb = [big.tile([128, HW], BF16, tag=f"k{m}", name=f"k{m}")
                    for m in range(NP)]
            # vT pair-tiles: [a][p, i, head, c] = v for kpix 128(2a+i)+p,
            # c==HD is the denominator ones column; head-dim padded to HD+2
            # so the DoubleRow pair-stride stays 16B-aligned
            vT_pair = [big.tile([128, 2, NH, HD + 2], FP8, tag=f"vT{a}",
                                name=f"vT{a}") for a in range(NA)]
            # attn pair-tiles: [g][p, i, n] = attn chans 128(2g+i)+p
            attn_pair = [big.tile([128, 2, HW], FP8, tag=f"at{g}",
                                  name=f"at{g}") for g in range(2)]
            po_tiles = {}    # hp -> [po_sub0, po_sub1]
            eT_all = {}      # hp -> [[eT pair tiles sub0], [sub1]]

            def qk_burst(hp, which):
                """DoubleRow q or k projection for pair hp (4 matmuls)."""
                dest, off, bc = ((q_sb, 0, hp) if which == "q"
                                 else (k_sb, HID, 4 + hp))
                pp = pbig.tile([128, HW], F32, tag="pb",
                               name=f"{which}_acc{hp}")
                for n in range(2):
                    for a in range(2):
                        nc.tensor.matmul(
                            pp[:, 512 * n:512 * (n + 1)],
                            lhsT=wqk_sb[a][:, :, off + 128 * hp:
                                           off + 128 * (hp + 1)],
                            rhs=h_pair[a][:, :, 512 * n:512 * (n + 1)],
                            start=(a == 0), stop=(a == 1), perf_mode=DR)
                nc.vector.tensor_scalar(
                    out=dest[hp], in0=pp[:, :],
                    scalar1=b_in_sb[:, bc:bc + 1], scalar2=None, op0=ADD)

            def v_one(p):
                """One v kpix-tile (pv pool, own PSUM banks), DoubleRow."""
                pp = pv.tile([128, 512], F32, tag="pv")
                for a in range(2):
                    nc.tensor.matmul(
                        pp[:, :],
                        lhsT=h_pair[a][:, :, 128 * p:128 * (p + 1)],
                        rhs=wv_sb[a][:, :, :],
                        start=(a == 0), stop=(a == 1), perf_mode=DR)
                nc.vector.tensor_copy(
                    out=vT_pair[p // 2][:, p % 2, :, 0:HD],
                    in_=pp[:, :].rearrange("a (nh c) -> a nh c", nh=NH))

            def out2_step(hp, a, subs=(0, 1)):
                """One DoubleRow kpix-pair accumulation of pair hp's out2."""
                if a == 0 and hp not in po_tiles:
                    po_tiles[hp] = [
                        pop.tile([HD + 1, HW], F32, tag="po",
                                 name=f"po{2 * hp + s}") for s in range(2)]
                eTs = eT_all[hp]
                for sub in subs:
                    head = 2 * hp + sub
                    po_t = po_tiles[hp][sub]
                    for n in range(2):
                        nc.tensor.matmul(
                            po_t[:, 512 * n:512 * (n + 1)],
                            lhsT=vT_pair[a][:, :, head, 0:HD + 1],
                            rhs=eTs[sub][a][:, :, 512 * n:512 * (n + 1)],
                            start=(a == 0), stop=(a == NA - 1), perf_mode=DR)

            def emit_logits_exp(hp, out2_of=None, fillers=None,
                                o2_defer=()):
                """Logits (bf16) + exp->fp8 for pair hp; out2 DR steps of
                pair out2_of ride at odd p (when an eT pair completes).
                sub1 exps at odd p run on Pool via the Schraudolph trick."""
                eTs = [[eTp.tile([128, 2, HW], FP8, bufs=2, tag=f"eT{sub}_{a}",
                                 name=f"eT{hp}_{sub}_{a}") for a in range(NA)]
                       for sub in range(2)]
                eT_all[hp] = eTs
                o2_queue = []
                for p in range(PT):
                    pls = []
                    for sub in range(2):
                        lo = 64 * sub
                        pl = pbig.tile([128, HW], F32, tag="pb",
                                       name=f"pl{hp}_{sub}_{p}")
                        for n in range(2):
                            nc.tensor.matmul(
                                pl[:, 512 * n:512 * (n + 1)],
                                lhsT=k_sb[hp][lo:lo + 64, 128 * p:128 * (p + 1)],
                                rhs=q_sb[hp][lo:lo + 64, 512 * n:512 * (n + 1)],
                                start=True, stop=True)
                        pls.append(pl)
                    for sub in range(2):
                        dst = eTs[sub][p // 2][:, p % 2, :]
                        if sub == 1 and p % 2 == 1 and hp == 3:
                            # Schraudolph bit-trick exp on DVE: relieves the
                            # ACT engine in the out2(3) chase region
                            nc.vector.tensor_scalar(
                                out=dst.bitcast(U8), in0=pls[sub][:, :],
                                scalar1=float(SCH_A), scalar2=float(SCH_B),
                                op0=MULT, op1=ADD)
                        else:
                            nc.scalar.activation(
                                out=dst, in_=pls[sub][:, :],
                                func=mybir.ActivationFunctionType.Exp,
                                scale=SCALE, bias=eshift_sb[:, :])
                    if out2_of is not None and p % 2 == 1:
                        a = p // 2
                        if a in o2_defer:
                            o2_queue.append(a)
                        else:
                            for qq in o2_queue:
                                out2_step(out2_of, qq)
                            o2_queue.clear()
                            out2_step(out2_of, a)
                    for f in (fillers or {}).get(p, []):
                        f()

            pair_state = {}

            def finish_den(hp, pe_bcast=False):
                """po -> attn (fp8, prescaled 2^-5, unnormalized) + den
                reciprocal. The odd head's partition hop rides a DMA that
                doesn't wait for the reciprocal; normalization is done in
                place afterwards by finish_mul."""
                eT_all.pop(hp)
                pos = po_tiles.pop(hp)
                g, i = hp // 2, hp % 2
                den8 = small.tile([2, HW], FP8, tag="den8", bufs=2,
                                  name=f"den8_{hp}")
                den2 = small.tile([2, HW], F32, tag="den2", bufs=2,
                                  name=f"den2_{hp}")
                rr2 = small.tile([2, HW], F32, tag="rr2", bufs=2,
                                 name=f"rr2_{hp}")
                den_eng = nc.gpsimd if pe_bcast else nc.sync
                hop_eng = nc.sync if pe_bcast else nc.gpsimd
                # one prescaled fp8 cast per head covers attn rows AND the
                # denominator row; the 1/32 prescale cancels exactly in
                # attn = au8 * (1/(den/32))
                au8s = []
                for sub in range(2):
                    au8 = small.tile([HD + 1, HW], FP8, tag="attnu",
                                     bufs=4, name=f"attnu{2 * hp + sub}")
                    nc.vector.tensor_scalar_mul(au8, pos[sub][:, :],
                                                float(ATT_PRE))
                    den_eng.dma_start(out=den8[sub:sub + 1, :],
                                      in_=au8[HD:HD + 1, :])
                    au8s.append(au8)
                # both heads' rows hop into the attn tile unnormalized
                # (before rb exists); normalize is then ONE in-place mul
                hop_eng.dma_start(out=attn_pair[g][HD:128, i, :],
                                  in_=au8s[1][0:HD, :])
                den_eng.dma_start(out=attn_pair[g][0:HD, i, :],
                                  in_=au8s[0][0:HD, :])
                # fp8->f32 widen on ACT (Identity + ~0 bias): parallel with
                # the DVE casts; then the DVE reciprocal
                nc.scalar.activation(
                    out=den2, in_=den8,
                    func=mybir.ActivationFunctionType.Identity,
                    bias=eps_sb[0:2, :], scale=1.0)
                nc.vector.reciprocal_approx_fast(out=rr2, in_=den2)
                if pe_bcast:
                    # one matmul pair broadcasts BOTH heads: indicator rows
                    # of selT pick rr2 row 0 for partitions 0-63, row 1 for
                    # 64-127
                    bb = pop.tile([128, HW], F32, tag="po", name=f"bb{hp}")
                    for n in range(2):
                        nc.tensor.matmul(
                            bb[:, 512 * n:512 * (n + 1)],
                            lhsT=ind2_sb[:, :],
                            rhs=rr2[0:2, 512 * n:512 * (n + 1)],
                            start=True, stop=True)
                    pair_state[hp] = (au8s[0], bb)
                else:
                    rr2b = small.tile([2, HW], BF16, tag="rr2b", bufs=2,
                                      name=f"rr2b_{hp}")
                    nc.vector.tensor_copy(out=rr2b, in_=rr2)
                    for sub in range(2):
                        nc.sync.dma_start(
                            out=rden_dram[2 * hp + sub:2 * hp + sub + 1, :],
                            in_=rr2b[sub:sub + 1, :])
                    pair_state[hp] = (au8s[0], None)

            def finish_mul(hp):
                """One in-place normalize mul over both heads' rows."""
                au8e, bb = pair_state.pop(hp)
                g, i = hp // 2, hp % 2
                if bb is None:
                    rbt = small.tile([128, HW], BF16, tag="rb", bufs=2,
                                     name=f"rb{hp}")
                    for sub in range(2):
                        bcast_ap = bass.AP(
                            tensor=rden_dram[:, :].tensor,
                            offset=(2 * hp + sub) * HW,
                            ap=[[0, HD], [1, HW]])
                        nc.sync.dma_start(out=rbt[64 * sub:64 * (sub + 1), :],
                                          in_=bcast_ap)
                    rb = rbt[:, :]
                    nc.gpsimd.tensor_mul(attn_pair[g][:, i, :],
                                         attn_pair[g][:, i, :], rb)
                else:
                    nc.vector.tensor_mul(attn_pair[g][:, i, :],
                                         attn_pair[g][:, i, :], bb[:, :])

            with tc.tile_pool(name="pbig", bufs=2, space="PSUM") as pbig:
                # GN per tile + qk0 DR accumulation rides along so the first
                # exp fires as soon as x1 lands.
                ppq0 = pbig.tile([128, HW], F32, tag="pb", name="q_acc0")
                ppk0 = pbig.tile([128, HW], F32, tag="pb", name="k_acc0")
                with tc.tile_pool(name="ps_gn", bufs=2, space="PSUM") as ps_gn:
                    eps_sb = small.tile([GPT, 1], F32, tag="eps_c", bufs=1)
                    nc.gpsimd.memset(eps_sb, float(EPS))
                    sq_scratch = small.tile([128, HW], BF16, tag="sqs",
                                            bufs=1)
                    sts = []

                    cp_scratch = small.tile([128, HW], BF16, tag="cps",
                                            bufs=1)

                    def gn_stats(t):
                        st = small.tile([128, 2], F32, tag=f"st{t}", bufs=1,
                                        name=f"st{t}")
                        sts.append(st)
                        nc.scalar.activation(
                            out=cp_scratch, in_=x_sb[t][:, :],
                            func=mybir.ActivationFunctionType.Copy,
                            accum_out=st[:, 0:1])
                        nc.vector.scalar_tensor_tensor(
                            out=sq_scratch, in0=x_sb[t][:, :], scalar=1.0,
                            in1=x_sb[t][:, :],
                            op0=mybir.AluOpType.bypass, op1=MULT,
                            accum_out=st[:, 1:2])

                    def gn_tile(t):
                        st = sts[t]
                        gpsum = ps_gn.tile([GPT, 2], F32, tag="gps")
                        nc.tensor.matmul(gpsum[:, :], lhsT=sel_sb[:, :],
                                         rhs=st[:, :], start=True, stop=True)
                        # grp cols: 0 rstd, 1 mean*rstd, 2 mean, 3 E[x^2]
                        grp = small.tile([GPT, 4], F32, tag="grp", bufs=2,
                                         name=f"grp{t}")
                        nc.vector.tensor_scalar_mul(grp[:, 2:4],
                                                    gpsum[:, 0:2], GN_INV)
                        nc.vector.tensor_mul(grp[:, 0:1], grp[:, 2:3],
                                             grp[:, 2:3])
                        nc.vector.tensor_sub(grp[:, 0:1], grp[:, 3:4],
                                             grp[:, 0:1])
                        nc.scalar.activation(
                            out=grp[:, 0:1], in_=grp[:, 0:1],
                            func=mybir.ActivationFunctionType.Sqrt,
                            bias=eps_sb[:, :], scale=1.0)
                        nc.vector.reciprocal(out=grp[:, 0:1], in_=grp[:, 0:1])
                        nc.vector.tensor_mul(grp[:, 1:2], grp[:, 2:3],
                                             grp[:, 0:1])
                        epsum = ps_gn.tile([128, 2], F32, tag="eps")
                        nc.tensor.matmul(epsum[:, :], lhsT=selT_sb[:, :],
                                         rhs=grp[:, 0:2], start=True,
                                         stop=True)
                        ga = small.tile([128, 1], F32, tag=f"ga{t}", bufs=1,
                                        name=f"ga{t}")
                        gd = small.tile([128, 1], F32, tag=f"gd{t}", bufs=1,
                                        name=f"gd{t}")
                        nc.vector.tensor_mul(ga[:, :], gamma_sb[:, t:t + 1],
                                             epsum[:, 0:1])
                        nc.vector.tensor_mul(gd[:, :], gamma_sb[:, t:t + 1],
                                             epsum[:, 1:2])
                        nc.vector.tensor_sub(gd[:, :], beta_sb[:, t:t + 1],
                                             gd[:, :])
                        nc.vector.tensor_scalar(
                            out=h_pair[t // 2][:, t % 2, :],
                            in0=x_sb[t][:, :],
                            scalar1=ga[:, :], scalar2=gd[:, :],
                            op0=MULT, op1=ADD)

                    # stats for t0/t1 first, then each tile's chain as
                    # soon as its stats are in — keeps grp(t) from queueing
                    # behind later tiles' big DVE reductions
                    gn_stats(0)
                    gn_stats(1)
                    for t in range(CT):
                        if t == 2:
                            gn_stats(2)
                        if t == 3:
                            gn_stats(3)
                        gn_tile(t)
                        if t % 2 == 1:
                            a = t // 2
                            for n in range(2):
                                nc.tensor.matmul(
                                    ppq0[:, 512 * n:512 * (n + 1)],
                                    lhsT=wqk_sb[a][:, :, 0:128],
                                    rhs=h_pair[a][:, :, 512 * n:512 * (n + 1)],
                                    start=(a == 0), stop=(a == 1),
                                    perf_mode=DR)
                                nc.tensor.matmul(
                                    ppk0[:, 512 * n:512 * (n + 1)],
                                    lhsT=wqk_sb[a][:, :, HID:HID + 128],
                                    rhs=h_pair[a][:, :, 512 * n:512 * (n + 1)],
                                    start=(a == 0), stop=(a == 1),
                                    perf_mode=DR)
                nc.vector.tensor_scalar(
                    out=q_sb[0], in0=ppq0[:, :],
                    scalar1=b_in_sb[:, 0:1], scalar2=None, op0=ADD)
                # k0 evict on ACT (idle pre-exp) so it runs parallel to the
                # q0 evict on DVE — both gate the first logits matmul.
                nc.scalar.activation(
                    out=k_sb[0], in_=ppk0[:, :],
                    func=mybir.ActivationFunctionType.Identity,
                    bias=b_in_sb[:, 4:5], scale=1.0)
                for a in range(NA):
                    for i in range(2):
                        nc.vector.memset(vT_pair[a][:, i, :, HD:HD + 1], 1.0)
                ind2_sb = const.tile([2, 128], F32)
                nc.sync.dma_start(out=ind2_sb,
                                  in_=selT_ext[GPT:GPT + 2, :])
                with tc.tile_pool(name="pv", bufs=2, space="PSUM") as pv:
                    emit_logits_exp(0, fillers={
                        0: [lambda: v_one(0)],
                        1: [lambda: v_one(1)],
                        2: [lambda: qk_burst(1, "q")],
                        3: [lambda: v_one(2)],
                        4: [lambda: v_one(3)],
                        5: [lambda: qk_burst(1, "k")],
                        6: [lambda: v_one(4), lambda: v_one(5)],
                        7: [lambda: v_one(6), lambda: v_one(7)],
                    })
                with tc.tile_pool(name="po", bufs=2, space="PSUM") as pop:
                    emit_logits_exp(1, out2_of=0, o2_defer=(1,), fillers={
                        2: [lambda: qk_burst(2, "q")],
                        5: [lambda: qk_burst(2, "k")],
                    })
                    finish_den(0)
                    emit_logits_exp(2, out2_of=1, o2_defer=(1,), fillers={
                        2: [lambda: qk_burst(3, "q")],
                        5: [lambda: qk_burst(3, "k")],
                    })
                    finish_den(1)
                    finish_mul(0)
                    emit_logits_exp(3, out2_of=2)
                    finish_den(2)
                    finish_mul(1)
                    # chase pair 3 sub-major: sub0's po finishes (and its
                    # au cast + den extract start) while sub1 still matmuls
                    for a in range(NA):
                        out2_step(3, a, subs=(0,))
                    for a in range(NA):
                        out2_step(3, a, subs=(1,))
                    finish_den(3, pe_bcast=True)
                    finish_mul(2)
                    finish_mul(3)

            # ---------- proj_out + bias + residual ----------
            # DoubleRow over chan-tile pairs: 2 accumulation steps per chunk.
            # pre = step a=0 (runs as soon as attn_pair[0] lands), fin = a=1
            # + evict + bf16 DMA out.
            with tc.tile_pool(name="ps_pout", bufs=2, space="PSUM") as ps_pout:
                def pout_chunk_pre(m, pps):
                    pp = ps_pout.tile([128, HW], F32, tag="pp",
                                      name=f"po_{m}")
                    pps[m] = pp
                    for n in range(2):
                        nc.tensor.matmul(
                            pp[:, 512 * n:512 * (n + 1)],
                            lhsT=wo_sb[0][:, :, 128 * m:128 * (m + 1)],
                            rhs=attn_pair[0][:, :, 512 * n:512 * (n + 1)],
                            start=True, stop=False, perf_mode=DR)

                def pout_chunk_fin(m, pps):
                    pp = pps.pop(m)
                    for n in range(2):
                        nc.tensor.matmul(
                            pp[:, 512 * n:512 * (n + 1)],
                            lhsT=wo_sb[1][:, :, 128 * m:128 * (m + 1)],
                            rhs=attn_pair[1][:, :, 512 * n:512 * (n + 1)],
                            start=False, stop=True, perf_mode=DR)
                    o_sb = small.tile([128, HW], BF16, tag="osb", bufs=2)
                    nc.vector.scalar_tensor_tensor(
                        out=o_sb, in0=pp[:, :],
                        scalar=b_out_sb[:, m:m + 1],
                        in1=x_sb[m][:, :],
                        op0=ADD, op1=ADD)
                    for hh in range(4):
                        deng = nc.gpsimd if (m + hh) % 2 else nc.sync
                        deng.dma_start(
                            out=out_ext[128 * m:128 * (m + 1),
                                        256 * hh:256 * (hh + 1)],
                            in_=o_sb[:, 256 * hh:256 * (hh + 1)])

                pps = {}
                for m in (0, 1):
                    pout_chunk_pre(m, pps)
                for m in (0, 1):
                    pout_chunk_fin(m, pps)
                for m in (2, 3):
                    pout_chunk_pre(m, pps)
                for m in (2, 3):
                    pout_chunk_fin(m, pps)
    return nc


def _install_ntff_hook():
    """The agent image's antenv lacks axon_hooks; synthesize it so
    run_bass_kernel_spmd(trace=True) can reach the NTFF profiler."""
    import types
    if "antenv.axon_hooks" in sys.modules:
        return
    mod = types.ModuleType("antenv.axon_hooks")
    mod._hook = None

    def set_axon_ntff_profile_hook(hook):
        mod._hook = hook

    def get_axon_ntff_profile_hook():
        return mod._hook

    mod.set_axon_ntff_profile_hook = set_axon_ntff_profile_hook
    mod.get_axon_ntff_profile_hook = get_axon_ntff_profile_hook
    sys.modules["antenv.axon_hooks"] = mod
    try:
        from trn_agent_boot.trn_boot import _ntff_profile_via_ctypes
        hook = _ntff_profile_via_ctypes("/opt/axon/libaxon_pjrt.so")
        if hook is not None:
            set_axon_ntff_profile_hook(hook)
    except Exception as e:  # degrade to no tracing
        print("ntff hook setup failed:", e)


_COMPILED = None


def _get_compiled():
    global _COMPILED
    if _COMPILED is None:
        nc = build_graph()
        nc.compile()
        _COMPILED = nc
    return _COMPILED


def _make_consts():
    # within any 128-channel tile, partition p belongs to local group p//16;
    # rows GPT/GPT+1 are even/odd-head indicator rows for the den broadcast
    sel = np.zeros((128, GPT), dtype=np.float32)
    selT = np.zeros((GPT + 2, 128), dtype=np.float32)
    for p in range(128):
        sel[p, p // GS] = 1.0
        selT[p // GS, p] = 1.0
    selT[GPT, 0:64] = 1.0
    selT[GPT + 1, 64:128] = 1.0
    return sel, selT


def _pm(v, cols):
    """[cols*128] vector -> partition-major [128, cols]."""
    return np.ascontiguousarray(v.reshape(cols, 128).T)


def _pack_pairs(w, m_dim):
    """w [m_dim, 512] -> [128, 2*2*m_dim] fp8: flat[a] [p, i, m] =
    w[m, 128*(2a+i)+p]."""
    wT = w.T.reshape(2, 2, 128, m_dim)           # [a, i, p, m]
    out = np.transpose(wT, (0, 2, 1, 3))          # [a, p, i, m]
    out = out.reshape(2, 128, 2 * m_dim)          # [a, p, i*m]
    out = np.concatenate([out[0], out[1]], axis=1)  # [p, a*i*m]
    return np.ascontiguousarray(out).astype(ml_dtypes.float8_e4m3)


def kernel(x, gamma, beta, w_in, b_in, w_out, b_out, _trace=False):
    x = np.asarray(x, dtype=np.float32)
    gamma = np.asarray(gamma, dtype=np.float32)
    beta = np.asarray(beta, dtype=np.float32)
    w_in = np.asarray(w_in, dtype=np.float32)
    b_in = np.asarray(b_in, dtype=np.float32)
    w_out = np.asarray(w_out, dtype=np.float32)
    b_out = np.asarray(b_out, dtype=np.float32)

    wqk8 = _pack_pairs(w_in[0:2 * HID], 2 * HID)   # q rows 0:512, k 512:1024
    wv8 = _pack_pairs(w_in[2 * HID:3 * HID], HID)
    wo8 = _pack_pairs(w_out, HID)
    sel, selT = _make_consts()
    # fold v-bias through proj_out: softmax rows sum to 1, so the attention
    # output is attn_raw + b_v exactly; w_out @ b_v + b_out replaces b_out.
    b_v = b_in[2 * HID:3 * HID]
    b_out_eff = b_out + w_out.astype(np.float64) @ b_v.astype(np.float64)
    b_out_eff = b_out_eff.astype(np.float32)
    cpack = np.zeros((128, 28), dtype=np.float32)
    cpack[:, 0:4] = _pm(gamma, CT)
    cpack[:, 4:8] = _pm(beta, CT)
    cpack[:, 8:16] = _pm(b_in[0:2 * HID], 8)
    cpack[:, 16:20] = _pm(b_out_eff, CT)
    cpack[:, 20:28] = sel
    common = {
        "wqk8": wqk8,
        "wv8": wv8,
        "wo8": wo8,
        "cpack": cpack,
        "gn_selT": selT,
    }
    in_maps = []
    for b in range(B):
        m = dict(common)
        m["x"] = np.ascontiguousarray(x[b].reshape(C, HW)).astype(
            ml_dtypes.bfloat16)
        in_maps.append(m)

    if _trace:
        _install_ntff_hook()
    nc = _get_compiled()
    res = run_bass_kernel_spmd(nc, in_maps, core_ids=list(range(B)),
                               trace=_trace)
    out = np.stack([np.asarray(res.results[b]["out"]).astype(np.float32)
                    .reshape(C, H, W) for b in range(B)])
    if _trace:
        return out, res
    return out


if __name__ == "__main__":
    rng = np.random.default_rng(0)
    inputs = {
        "x": rng.standard_normal((B, C, H, W), dtype=np.float32),
        "gamma": np.ones(C, dtype=np.float32),
        "beta": np.zeros(C, dtype=np.float32),
        "w_in": (rng.standard_normal((3 * HID, C), dtype=np.float32)
                 / np.sqrt(C)),
        "b_in": np.zeros(3 * HID, dtype=np.float32),
        "w_out": (rng.standard_normal((C, HID), dtype=np.float32)
                  / np.sqrt(HID)),
        "b_out": np.zeros(C, dtype=np.float32),
    }
    out = kernel(**inputs)
    print("kernel ran, out shape", out.shape)


# revision 16
# speedup vs baseline: 1.0177x; 1.0054x over previous
"""Trainium2 Bass kernel for nn_AttentionBlock (GroupNorm + 8-head attention
block on [8, 512, 32, 32], residual).

Sharding: pure data-parallel over batch B=8 across the 8 NeuronCores — one
batch element per core, weights replicated, zero collectives.

v5 = v4 (fp8e4 DoubleRow matmuls) + schedule/latency rework:
  - Head: x tiles stream in halves across both DMA queues before any other
    load; GN sums ride the idle ACT engine (Copy+accum), sum-of-squares on
    DVE, and the per-group algebra chain is DVE-resident (fewer cross-engine
    sem hops). Memsets issue after the loads.
  - Denominators: the den row is DMA'd straight out of the po PSUM (f32, no
    bf16 staging); reciprocals are scaled by 32 and broadcast per head as
    bf16 via the DRAM round trip (pairs 0-2) or a PE ones-matmul into a
    [128,1024] PSUM tile (pair 3, lowest latency).
  - attn: po rows are cast PSUM->fp8 with a 2^-5 prescale (unnormalized
    values reach ~733 > fp8 max), the odd head's rows hop partitions by DMA
    *before* the reciprocal arrives, and normalization happens in place
    (attn *= 32/den). This frees po banks at cast time and removes the au65
    staging tiles of v3/v4.
  - exp: odd-p sub1 tiles compute on the Pool engine via a Schraudolph
    bit-trick (uint8 = 1.4427*l + 32.46 IS the fp8e4m3 bit pattern of
    exp(l/8 - 2), ~3% rel err) so the ACT engine stops pacing the
    logits->exp->out2 pipeline. Attention carries ~7.6% of the output norm,
    so these approximations cost ~0.3% end-to-end (measured 6.3e-3 total,
    tolerance 2e-2).
  - proj_out: DoubleRow over chan-tile pairs, first-half accumulations run
    during the attention tail, output DMA'd as bf16.
"""
import sys

sys.path.insert(0, "/opt/trn_rl_repo")

import numpy as np
import ml_dtypes

import concourse.bass as bass
import concourse.bacc as bacc
import concourse.tile as tile
from concourse import mybir
from concourse.bass_utils import run_bass_kernel_spmd

F32 = mybir.dt.float32
BF16 = mybir.dt.bfloat16
FP8 = mybir.dt.float8e4
U8 = mybir.dt.uint8
ADD = mybir.AluOpType.add
MULT = mybir.AluOpType.mult
SUB = mybir.AluOpType.subtract
DR = mybir.MatmulPerfMode.DoubleRow

B, C, H, W = 8, 512, 32, 32
HW = H * W       # 1024
NG = 32          # groups
GS = C // NG     # 16 channels per group
NH = 8           # heads
HD = 64          # head dim
HID = NH * HD    # 512
NP = NH // 2     # 4 head pairs
EPS = 1e-6
SCALE = 1.0 / float(np.sqrt(HD))  # 0.125
EXP_SHIFT = -2.0  # exp(scale*l + shift): keeps e' under fp8e4 max (240)
ATT_PRE = 1.0 / 32  # prescale for the unnormalized po->fp8 cast
# Schraudolph fp8e4m3 bit-pattern exp: u8 = SCH_A*logit + SCH_B
SCH_A = 8.0 / np.log(2.0) * SCALE            # 1.44270
SCH_B = 8.0 * (np.log2(np.e) * EXP_SHIFT + 7.0) - 0.458
CT = C // 128    # 4 channel partition-tiles
PT = HW // 128   # 8 pixel partition-tiles
NA = PT // 2     # 4 kpix-tile pairs (DoubleRow accumulation steps)
GPT = NG // CT   # 8 groups per channel-tile
GN_INV = 1.0 / (GS * HW)          # 1/16384


def build_graph():
    nc = bacc.Bacc("TRN2", num_devices=8)

    x_ext = nc.declare_dram_parameter("x", [C, HW], BF16, isOutput=False)
    # fp8 pair-packed weights: [a][p, i, m] with contraction chan 128(2a+i)+p
    wqk_ext = nc.declare_dram_parameter("wqk8", [128, 2 * 2 * 1024], FP8,
                                        isOutput=False)
    wv_ext = nc.declare_dram_parameter("wv8", [128, 2 * 2 * 512], FP8,
                                       isOutput=False)
    wo_ext = nc.declare_dram_parameter("wo8", [128, 2 * 2 * 512], FP8,
                                       isOutput=False)
    # packed [128, 28] consts: 0:4 gamma, 4:8 beta, 8:16 b_in(q,k),
    # 16:20 b_out_eff, 20:28 gn_sel
    cpack_ext = nc.declare_dram_parameter("cpack", [128, 28], F32, isOutput=False)
    selT_ext = nc.declare_dram_parameter("gn_selT", [GPT + 2, 128], F32,
                                        isOutput=False)
    out_ext = nc.declare_dram_parameter("out", [C, HW], BF16, isOutput=True)

    rden_dram = nc.dram_tensor("rden_scratch", [NH, HW], BF16)

    with tile.TileContext(nc) as tc:
        with (
            tc.tile_pool(name="const", bufs=1) as const,
            tc.tile_pool(name="big", bufs=1) as big,
            tc.tile_pool(name="eT", bufs=1) as eTp,
            tc.tile_pool(name="small", bufs=2) as small,
        ):
            # ---------- loads: x0/x1 in halves across both queues first
            # (they gate the GN chain), then x2/x3, consts, weights ----------
            x_sb = [big.tile([128, HW], BF16, tag=f"x{t}", name=f"x{t}")
                    for t in range(CT)]
            for t in (0, 1):
                nc.gpsimd.dma_start(out=x_sb[t][:, 0:512],
                                    in_=x_ext[128 * t:128 * (t + 1), 0:512])
                nc.sync.dma_start(out=x_sb[t][:, 512:1024],
                                  in_=x_ext[128 * t:128 * (t + 1), 512:1024])
            nc.gpsimd.dma_start(out=x_sb[2], in_=x_ext[256:384, :])
            nc.sync.dma_start(out=x_sb[3], in_=x_ext[384:512, :])
            cpack_sb = const.tile([128, 28], F32)
            nc.gpsimd.dma_start(out=cpack_sb, in_=cpack_ext[:, :])
            selT_sb = const.tile([GPT, 128], F32)
            nc.gpsimd.dma_start(out=selT_sb, in_=selT_ext[0:GPT, :])
            gamma_sb = cpack_sb[:, 0:4]
            beta_sb = cpack_sb[:, 4:8]
            b_in_sb = cpack_sb[:, 8:16]
            b_out_sb = cpack_sb[:, 16:20]
            sel_sb = cpack_sb[:, 20:28]
            # fp8 weight pair-tiles
            wqk_sb = [big.tile([128, 2, 2 * HID], FP8, tag=f"wqk{a}",
                               name=f"wqk{a}") for a in range(2)]
            for a in range(2):
                nc.sync.dma_start(
                    out=wqk_sb[a][:, :, :],
                    in_=wqk_ext[:, 2 * HID * 2 * a:2 * HID * 2 * (a + 1)]
                    .rearrange("p (i m) -> p i m", i=2))
            wv_sb = [big.tile([128, 2, HID], FP8, tag=f"wv{a}",
                              name=f"wv{a}") for a in range(2)]
            for a in range(2):
                nc.sync.dma_start(
                    out=wv_sb[a][:, :, :],
                    in_=wv_ext[:, HID * 2 * a:HID * 2 * (a + 1)]
                    .rearrange("p (i m) -> p i m", i=2))
            wo_sb = [big.tile([128, 2, HID], FP8, tag=f"wo{a}",
                              name=f"wo{a}") for a in range(2)]
            for a in range(2):
                nc.sync.dma_start(
                    out=wo_sb[a][:, :, :],
                    in_=wo_ext[:, HID * 2 * a:HID * 2 * (a + 1)]
                    .rearrange("p (i m) -> p i m", i=2))
            eshift_sb = const.tile([128, 1], F32)
            nc.vector.memset(eshift_sb, float(EXP_SHIFT))
            one_sb = const.tile([128, 1], F32)
            nc.vector.memset(one_sb, 1.0)
            # dummy ops hoist the ACT table loads (Sqrt/Identity and Exp
            # sets) into the idle pre-x window instead of the GN/exp path
            tl_scratch = small.tile([128, 1], F32, tag="tls", bufs=1)
            nc.scalar.activation(out=tl_scratch, in_=one_sb,
                                 func=mybir.ActivationFunctionType.Sqrt,
                                 bias=one_sb[:, :], scale=1.0)
            nc.scalar.activation(out=tl_scratch, in_=one_sb,
                                 func=mybir.ActivationFunctionType.Exp,
                                 scale=1.0, bias=one_sb[:, :])
            nc.scalar.activation(out=tl_scratch, in_=one_sb,
                                 func=mybir.ActivationFunctionType.Copy)

            # ---------- SBUF state ----------
            # h in fp8 pair-tiles: h_pair[a][:, i, :] = GN output chan-tile 2a+i
            h_pair = [big.tile([128, 2, HW], FP8, tag=f"h{a}", name=f"h{a}")
                      for a in range(2)]
            q_sb = [big.tile([128, HW], BF16, tag=f"q{m}", name=f"q{m}")
                    for m in range(NP)]
            k_sb = [big.tile([128, HW], BF16, tag=f"k{m}", name=f"k{m}")
                    for m in range(NP)]
            # vT pair-tiles: [a][p, i, head, c] = v for kpix 128(2a+i)+p,
            # c==HD is the denominator ones column; head-dim padded to HD+2
            # so the DoubleRow pair-stride stays 16B-aligned
            vT_pair = [big.tile([128, 2, NH, HD + 2], FP8, tag=f"vT{a}",
                                name=f"vT{a}") for a in range(NA)]
            # attn pair-tiles: [g][p, i, n] = attn chans 128(2g+i)+p
            attn_pair = [big.tile([128, 2, HW], FP8, tag=f"at{g}",
                                  name=f"at{g}") for g in range(2)]
            po_tiles = {}    # hp -> [po_sub0, po_sub1]
            eT_all = {}      # hp -> [[eT pair tiles sub0], [sub1]]

            def qk_burst(hp, which):
                """DoubleRow q or k projection for pair hp (4 matmuls)."""
                dest, off, bc = ((q_sb, 0, hp) if which == "q"
                                 else (k_sb, HID, 4 + hp))
                pp = pbig.tile([128, HW], F32, tag="pb",
                               name=f"{which}_acc{hp}")
                for n in range(2):
                    for a in range(2):
                        nc.tensor.matmul(
                            pp[:, 512 * n:512 * (n + 1)],
                            lhsT=wqk_sb[a][:, :, off + 128 * hp:
                                           off + 128 * (hp + 1)],
                            rhs=h_pair[a][:, :, 512 * n:512 * (n + 1)],
                            start=(a == 0), stop=(a == 1), perf_mode=DR)
                nc.vector.tensor_scalar(
                    out=dest[hp], in0=pp[:, :],
                    scalar1=b_in_sb[:, bc:bc + 1], scalar2=None, op0=ADD)

            def v_one(p):
                """One v kpix-tile (pv pool, own PSUM banks), DoubleRow."""
                pp = pv.tile([128, 512], F32, tag="pv")
                for a in range(2):
                    nc.tensor.matmul(
                        pp[:, :],
                        lhsT=h_pair[a][:, :, 128 * p:128 * (p + 1)],
                        rhs=wv_sb[a][:, :, :],
                        start=(a == 0), stop=(a == 1), perf_mode=DR)
                nc.vector.tensor_copy(
                    out=vT_pair[p // 2][:, p % 2, :, 0:HD],
                    in_=pp[:, :].rearrange("a (nh c) -> a nh c", nh=NH))

            def out2_step(hp, a, subs=(0, 1)):
                """One DoubleRow kpix-pair accumulation of pair hp's out2."""
                if a == 0 and hp not in po_tiles:
                    po_tiles[hp] = [
                        pop.tile([HD + 1, HW], F32, tag="po",
                                 name=f"po{2 * hp + s}") for s in range(2)]
                eTs = eT_all[hp]
                for sub in subs:
                    head = 2 * hp + sub
                    po_t = po_tiles[hp][sub]
                    for n in range(2):
                        nc.tensor.matmul(
                            po_t[:, 512 * n:512 * (n + 1)],
                            lhsT=vT_pair[a][:, :, head, 0:HD + 1],
                            rhs=eTs[sub][a][:, :, 512 * n:512 * (n + 1)],
                            start=(a == 0), stop=(a == NA - 1), perf_mode=DR)

            def emit_logits_exp(hp, out2_of=None, fillers=None,
                                o2_defer=()):
                """Logits (bf16) + exp->fp8 for pair hp; out2 DR steps of
                pair out2_of ride at odd p (when an eT pair completes).
                sub1 exps at odd p run on Pool via the Schraudolph trick."""
                eTs = [[eTp.tile([128, 2, HW], FP8, bufs=2, tag=f"eT{sub}_{a}",
                                 name=f"eT{hp}_{sub}_{a}") for a in range(NA)]
                       for sub in range(2)]
                eT_all[hp] = eTs
                o2_queue = []
                for p in range(PT):
                    pls = []
                    for sub in range(2):
                        lo = 64 * sub
                        pl = pbig.tile([128, HW], F32, tag="pb",
                                       name=f"pl{hp}_{sub}_{p}")
                        for n in range(2):
                            nc.tensor.matmul(
                                pl[:, 512 * n:512 * (n + 1)],
                                lhsT=k_sb[hp][lo:lo + 64, 128 * p:128 * (p + 1)],
                                rhs=q_sb[hp][lo:lo + 64, 512 * n:512 * (n + 1)],
                                start=True, stop=True)
                        pls.append(pl)
                    for sub in range(2):
                        dst = eTs[sub][p // 2][:, p % 2, :]
                        if sub == 1 and p % 2 == 1 and hp == 3:
                            # Schraudolph bit-trick exp on DVE: relieves the
                            # ACT engine in the out2(3) chase region
                            nc.vector.tensor_scalar(
                                out=dst.bitcast(U8), in0=pls[sub][:, :],
                                scalar1=float(SCH_A), scalar2=float(SCH_B),
                                op0=MULT, op1=ADD)
                        else:
                            nc.scalar.activation(
                                out=dst, in_=pls[sub][:, :],
                                func=mybir.ActivationFunctionType.Exp,
                                scale=SCALE, bias=eshift_sb[:, :])
                    if out2_of is not None and p % 2 == 1:
                        a = p // 2
                        if a in o2_defer:
                            o2_queue.append(a)
                        else:
                            for qq in o2_queue:
                                out2_step(out2_of, qq)
                            o2_queue.clear()
                            out2_step(out2_of, a)
                    for f in (fillers or {}).get(p, []):
                        f()

            pair_state = {}

            def finish_den(hp, pe_bcast=False):
                """po -> attn (fp8, prescaled 2^-5, unnormalized) + den
                reciprocal. The odd head's partition hop rides a DMA that
                doesn't wait for the reciprocal; normalization is done in
                place afterwards by finish_mul."""
                eT_all.pop(hp)
                pos = po_tiles.pop(hp)
                g, i = hp // 2, hp % 2
                den8 = small.tile([2, HW], FP8, tag="den8", bufs=2,
                                  name=f"den8_{hp}")
                den2 = small.tile([2, HW], F32, tag="den2", bufs=2,
                                  name=f"den2_{hp}")
                rr2 = small.tile([2, HW], F32, tag="rr2", bufs=2,
                                 name=f"rr2_{hp}")
                den_eng = nc.gpsimd if pe_bcast else nc.sync
                hop_eng = nc.sync if pe_bcast else nc.gpsimd
                # one prescaled fp8 cast per head covers attn rows AND the
                # denominator row; the 1/32 prescale cancels exactly in
                # attn = au8 * (1/(den/32))
                au8s = []
                for sub in range(2):
                    au8 = small.tile([HD + 1, HW], FP8, tag="attnu",
                                     bufs=4, name=f"attnu{2 * hp + sub}")
                    nc.vector.tensor_scalar_mul(au8, pos[sub][:, :],
                                                float(ATT_PRE))
                    den_eng.dma_start(out=den8[sub:sub + 1, :],
                                      in_=au8[HD:HD + 1, :])
                    au8s.append(au8)
                # both heads' rows hop into the attn tile unnormalized
                # (before rb exists); normalize is then ONE in-place mul
                hop_eng.dma_start(out=attn_pair[g][HD:128, i, :],
                                  in_=au8s[1][0:HD, :])
                den_eng.dma_start(out=attn_pair[g][0:HD, i, :],
                                  in_=au8s[0][0:HD, :])
                # fp8->f32 widen on ACT (Identity + ~0 bias): parallel with
                # the DVE casts; then the DVE reciprocal
                nc.scalar.activation(
                    out=den2, in_=den8,
                    func=mybir.ActivationFunctionType.Identity,
                    bias=eps_sb[0:2, :], scale=1.0)
                nc.vector.reciprocal_approx_fast(out=rr2, in_=den2)
                if pe_bcast:
                    # one matmul pair broadcasts BOTH heads: indicator rows
                    # of selT pick rr2 row 0 for partitions 0-63, row 1 for
                    # 64-127
                    bb = pop.tile([128, HW], F32, tag="po", name=f"bb{hp}")
                    for n in range(2):
                        nc.tensor.matmul(
                            bb[:, 512 * n:512 * (n + 1)],
                            lhsT=ind2_sb[:, :],
                            rhs=rr2[0:2, 512 * n:512 * (n + 1)],
                            start=True, stop=True)
                    pair_state[hp] = (au8s[0], bb)
                else:
                    rr2b = small.tile([2, HW], BF16, tag="rr2b", bufs=2,
                                      name=f"rr2b_{hp}")
                    nc.vector.tensor_copy(out=rr2b, in_=rr2)
                    for sub in range(2):
                        nc.sync.dma_start(
                            out=rden_dram[2 * hp + sub:2 * hp + sub + 1, :],
                            in_=rr2b[sub:sub + 1, :])
                    pair_state[hp] = (au8s[0], None)

            def finish_mul(hp):
                """One in-place normalize mul over both heads' rows."""
                au8e, bb = pair_state.pop(hp)
                g, i = hp // 2, hp % 2
                if bb is None:
                    rbt = small.tile([128, HW], BF16, tag="rb", bufs=2,
                                     name=f"rb{hp}")
                    for sub in range(2):
                        bcast_ap = bass.AP(
                            tensor=rden_dram[:, :].tensor,
                            offset=(2 * hp + sub) * HW,
                            ap=[[0, HD], [1, HW]])
                        nc.sync.dma_start(out=rbt[64 * sub:64 * (sub + 1), :],
                                          in_=bcast_ap)
                    rb = rbt[:, :]
                    nc.gpsimd.tensor_mul(attn_pair[g][:, i, :],
                                         attn_pair[g][:, i, :], rb)
                else:
                    nc.vector.tensor_mul(attn_pair[g][:, i, :],
                                         attn_pair[g][:, i, :], bb[:, :])

            with tc.tile_pool(name="pbig", bufs=2, space="PSUM") as pbig:
                # GN per tile + qk0 DR accumulation rides along so the first
                # exp fires as soon as x1 lands.
                ppq0 = pbig.tile([128, HW], F32, tag="pb", name="q_acc0")
                ppk0 = pbig.tile([128, HW], F32, tag="pb", name="k_acc0")
                with tc.tile_pool(name="ps_gn", bufs=2, space="PSUM") as ps_gn:
                    eps_sb = small.tile([GPT, 1], F32, tag="eps_c", bufs=1)
                    nc.gpsimd.memset(eps_sb, float(EPS))
                    sq_scratch = small.tile([128, HW], BF16, tag="sqs",
                                            bufs=1)
                    sts = []

                    cp_scratch = small.tile([128, HW], BF16, tag="cps",
                                            bufs=1)

                    def gn_stats(t):
                        st = small.tile([128, 2], F32, tag=f"st{t}", bufs=1,
                                        name=f"st{t}")
                        sts.append(st)
                        nc.scalar.activation(
                            out=cp_scratch, in_=x_sb[t][:, :],
                            func=mybir.ActivationFunctionType.Copy,
                            accum_out=st[:, 0:1])
                        nc.vector.scalar_tensor_tensor(
                            out=sq_scratch, in0=x_sb[t][:, :], scalar=1.0,
                            in1=x_sb[t][:, :],
                            op0=mybir.AluOpType.bypass, op1=MULT,
                            accum_out=st[:, 1:2])

                    def gn_tile(t):
                        st = sts[t]
                        gpsum = ps_gn.tile([GPT, 2], F32, tag="gps")
                        nc.tensor.matmul(gpsum[:, :], lhsT=sel_sb[:, :],
                                         rhs=st[:, :], start=True, stop=True)
                        # grp cols: 0 rstd, 1 mean*rstd, 2 mean, 3 E[x^2]
                        grp = small.tile([GPT, 4], F32, tag="grp", bufs=2,
                                         name=f"grp{t}")
                        nc.vector.tensor_scalar_mul(grp[:, 2:4],
                                                    gpsum[:, 0:2], GN_INV)
                        nc.vector.tensor_mul(grp[:, 0:1], grp[:, 2:3],
                                             grp[:, 2:3])
                        nc.vector.tensor_sub(grp[:, 0:1], grp[:, 3:4],
                                             grp[:, 0:1])
                        nc.scalar.activation(
                            out=grp[:, 0:1], in_=grp[:, 0:1],
                            func=mybir.ActivationFunctionType.Sqrt,
                            bias=eps_sb[:, :], scale=1.0)
                        nc.vector.reciprocal(out=grp[:, 0:1], in_=grp[:, 0:1])
                        nc.vector.tensor_mul(grp[:, 1:2], grp[:, 2:3],
                                             grp[:, 0:1])
                        epsum = ps_gn.tile([128, 2], F32, tag="eps")
                        nc.tensor.matmul(epsum[:, :], lhsT=selT_sb[:, :],
                                         rhs=grp[:, 0:2], start=True,
                                         stop=True)
                        ga = small.tile([128, 1], F32, tag=f"ga{t}", bufs=1,
                                        name=f"ga{t}")
                        gd = small.tile([128, 1], F32, tag=f"gd{t}", bufs=1,
                                        name=f"gd{t}")
                        nc.vector.tensor_mul(ga[:, :], gamma_sb[:, t:t + 1],
                                             epsum[:, 0:1])
                        nc.vector.tensor_mul(gd[:, :], gamma_sb[:, t:t + 1],
                                             epsum[:, 1:2])
                        nc.vector.tensor_sub(gd[:, :], beta_sb[:, t:t + 1],
                                             gd[:, :])
                        nc.vector.tensor_scalar(
                            out=h_pair[t // 2][:, t % 2, :],
                            in0=x_sb[t][:, :],
                            scalar1=ga[:, :], scalar2=gd[:, :],
                            op0=MULT, op1=ADD)

                    # stats for t0/t1 first, then each tile's chain as
                    # soon as its stats are in — keeps grp(t) from queueing
                    # behind later tiles' big DVE reductions
                    gn_stats(0)
                    gn_stats(1)
                    for t in range(CT):
                        if t == 2:
                            gn_stats(2)
                        if t == 3:
                            gn_stats(3)
                        gn_tile(t)
                        if t % 2 == 1:
                            a = t // 2
                            for n in range(2):
                                nc.tensor.matmul(
                                    ppq0[:, 512 * n:512 * (n + 1)],
                                    lhsT=wqk_sb[a][:, :, 0:128],
                                    rhs=h_pair[a][:, :, 512 * n:512 * (n + 1)],
                                    start=(a == 0), stop=(a == 1),
                                    perf_mode=DR)
                                nc.tensor.matmul(
                                    ppk0[:, 512 * n:512 * (n + 1)],
                                    lhsT=wqk_sb[a][:, :, HID:HID + 128],
                                    rhs=h_pair[a][:, :, 512 * n:512 * (n + 1)],
                                    start=(a == 0), stop=(a == 1),
                                    perf_mode=DR)
                nc.vector.tensor_scalar(
                    out=q_sb[0], in0=ppq0[:, :],
                    scalar1=b_in_sb[:, 0:1], scalar2=None, op0=ADD)
                # k0 evict on ACT (idle pre-exp) so it runs parallel to the
                # q0 evict on DVE — both gate the first logits matmul.
                nc.scalar.activation(
                    out=k_sb[0], in_=ppk0[:, :],
                    func=mybir.ActivationFunctionType.Identity,
                    bias=b_in_sb[:, 4:5], scale=1.0)
                for a in range(NA):
                    for i in range(2):
                        nc.vector.memset(vT_pair[a][:, i, :, HD:HD + 1], 1.0)
                ind2_sb = const.tile([2, 128], F32)
                nc.sync.dma_start(out=ind2_sb,
                                  in_=selT_ext[GPT:GPT + 2, :])
                with tc.tile_pool(name="pv", bufs=2, space="PSUM") as pv:
                    emit_logits_exp(0, fillers={
                        0: [lambda: v_one(0)],
                        1: [lambda: v_one(1)],
                        2: [lambda: qk_burst(1, "q")],
                        3: [lambda: v_one(2)],
                        4: [lambda: v_one(3)],
                        5: [lambda: qk_burst(1, "k")],
                        6: [lambda: v_one(4), lambda: v_one(5)],
                        7: [lambda: v_one(6), lambda: v_one(7)],
                    })
                with tc.tile_pool(name="po", bufs=2, space="PSUM") as pop:
                    emit_logits_exp(1, out2_of=0, o2_defer=(1,), fillers={
                        2: [lambda: qk_burst(2, "q")],
                        5: [lambda: qk_burst(2, "k")],
                    })
                    finish_den(0)
                    emit_logits_exp(2, out2_of=1, o2_defer=(1,), fillers={
                        2: [lambda: qk_burst(3, "q")],
                        5: [lambda: qk_burst(3, "k")],
                    })
                    finish_den(1)
                    finish_mul(0)
                    emit_logits_exp(3, out2_of=2)
                    finish_den(2)
                    finish_mul(1)
                    # chase pair 3 sub-major: sub0's po finishes (and its
                    # au cast + den extract start) while sub1 still matmuls
                    for a in range(NA):
                        out2_step(3, a, subs=(0,))
                    for a in range(NA):
                        out2_step(3, a, subs=(1,))
                    finish_den(3, pe_bcast=True)
                    finish_mul(2)
                    finish_mul(3)

            # ---------- proj_out + bias + residual ----------
            # DoubleRow over chan-tile pairs: 2 accumulation steps per chunk.
            # pre = step a=0 (runs as soon as attn_pair[0] lands), fin = a=1
            # + evict + bf16 DMA out.
            with tc.tile_pool(name="ps_pout", bufs=8, space="PSUM") as ps_pout:
                # ring of 8: every chunk's first DoubleRow accumulation can
                # run during the attention tail; after mul(3) only the 8
                # closing matmuls + evicts remain
                def pout_chunk_pre(m, n, pps):
                    pp = ps_pout.tile([128, 512], F32, tag="pp",
                                      name=f"po_{m}_{n}")
                    pps[(m, n)] = pp
                    nc.tensor.matmul(
                        pp[:, :],
                        lhsT=wo_sb[0][:, :, 128 * m:128 * (m + 1)],
                        rhs=attn_pair[0][:, :, 512 * n:512 * (n + 1)],
                        start=True, stop=False, perf_mode=DR)

                def pout_chunk_fin(m, n, pps):
                    pp = pps.pop((m, n))
                    nc.tensor.matmul(
                        pp[:, :],
                        lhsT=wo_sb[1][:, :, 128 * m:128 * (m + 1)],
                        rhs=attn_pair[1][:, :, 512 * n:512 * (n + 1)],
                        start=False, stop=True, perf_mode=DR)
                    o_sb = small.tile([128, 512], BF16, tag="osb", bufs=4)
                    nc.vector.scalar_tensor_tensor(
                        out=o_sb, in0=pp[:, :],
                        scalar=b_out_sb[:, m:m + 1],
                        in1=x_sb[m][:, 512 * n:512 * (n + 1)],
                        op0=ADD, op1=ADD)
                    for hh in range(2):
                        deng = nc.gpsimd if (2 * m + n + hh) % 2 else nc.sync
                        deng.dma_start(
                            out=out_ext[128 * m:128 * (m + 1),
                                        512 * n + 256 * hh:
                                        512 * n + 256 * (hh + 1)],
                            in_=o_sb[:, 256 * hh:256 * (hh + 1)])

                pps = {}
                chunks = [(m, n) for n in range(2) for m in range(CT)]
                for m, n in chunks:
                    pout_chunk_pre(m, n, pps)
                for m, n in chunks:
                    pout_chunk_fin(m, n, pps)
    return nc


def _install_ntff_hook():
    """The agent image's antenv lacks axon_hooks; synthesize it so
    run_bass_kernel_spmd(trace=True) can reach the NTFF profiler."""
    import types
    if "antenv.axon_hooks" in sys.modules:
        return
    mod = types.ModuleType("antenv.axon_hooks")
    mod._hook = None

    def set_axon_ntff_profile_hook(hook):
        mod._hook = hook

    def get_axon_ntff_profile_hook():
        return mod._hook

    mod.set_axon_ntff_profile_hook = set_axon_ntff_profile_hook
    mod.get_axon_ntff_profile_hook = get_axon_ntff_profile_hook
    sys.modules["antenv.axon_hooks"] = mod
    try:
        from trn_agent_boot.trn_boot import _ntff_profile_via_ctypes
        hook = _ntff_profile_via_ctypes("/opt/axon/libaxon_pjrt.so")
        if hook is not None:
            set_axon_ntff_profile_hook(hook)
    except Exception as e:  # degrade to no tracing
        print("ntff hook setup failed:", e)


_COMPILED = None


def _get_compiled():
    global _COMPILED
    if _COMPILED is None:
        nc = build_graph()
        nc.compile()
        _COMPILED = nc
    return _COMPILED


def _make_consts():
    # within any 128-channel tile, partition p belongs to local group p//16;
    # rows GPT/GPT+1 are even/odd-head indicator rows for the den broadcast
    sel = np.zeros((128, GPT), dtype=np.float32)
    selT = np.zeros((GPT + 2, 128), dtype=np.float32)
    for p in range(128):
        sel[p, p // GS] = 1.0
        selT[p // GS, p] = 1.0
    selT[GPT, 0:64] = 1.0
    selT[GPT + 1, 64:128] = 1.0
    return sel, selT


def _pm(v, cols):
    """[cols*128] vector -> partition-major [128, cols]."""
    return np.ascontiguousarray(v.reshape(cols, 128).T)


def _pack_pairs(w, m_dim):
    """w [m_dim, 512] -> [128, 2*2*m_dim] fp8: flat[a] [p, i, m] =
    w[m, 128*(2a+i)+p]."""
    wT = w.T.reshape(2, 2, 128, m_dim)           # [a, i, p, m]
    out = np.transpose(wT, (0, 2, 1, 3))          # [a, p, i, m]
    out = out.reshape(2, 128, 2 * m_dim)          # [a, p, i*m]
    out = np.concatenate([out[0], out[1]], axis=1)  # [p, a*i*m]
    return np.ascontiguousarray(out).astype(ml_dtypes.float8_e4m3)


def kernel(x, gamma, beta, w_in, b_in, w_out, b_out, _trace=False):
    x = np.asarray(x, dtype=np.float32)
    gamma = np.asarray(gamma, dtype=np.float32)
    beta = np.asarray(beta, dtype=np.float32)
    w_in = np.asarray(w_in, dtype=np.float32)
    b_in = np.asarray(b_in, dtype=np.float32)
    w_out = np.asarray(w_out, dtype=np.float32)
    b_out = np.asarray(b_out, dtype=np.float32)

    wqk8 = _pack_pairs(w_in[0:2 * HID], 2 * HID)   # q rows 0:512, k 512:1024
    wv8 = _pack_pairs(w_in[2 * HID:3 * HID], HID)
    wo8 = _pack_pairs(w_out, HID)
    sel, selT = _make_consts()
    # fold v-bias through proj_out: softmax rows sum to 1, so the attention
    # output is attn_raw + b_v exactly; w_out @ b_v + b_out replaces b_out.
    b_v = b_in[2 * HID:3 * HID]
    b_out_eff = b_out + w_out.astype(np.float64) @ b_v.astype(np.float64)
    b_out_eff = b_out_eff.astype(np.float32)
    cpack = np.zeros((128, 28), dtype=np.float32)
    cpack[:, 0:4] = _pm(gamma, CT)
    cpack[:, 4:8] = _pm(beta, CT)
    cpack[:, 8:16] = _pm(b_in[0:2 * HID], 8)
    cpack[:, 16:20] = _pm(b_out_eff, CT)
    cpack[:, 20:28] = sel
    common = {
        "wqk8": wqk8,
        "wv8": wv8,
        "wo8": wo8,
        "cpack": cpack,
        "gn_selT": selT,
    }
    in_maps = []
    for b in range(B):
        m = dict(common)
        m["x"] = np.ascontiguousarray(x[b].reshape(C, HW)).astype(
            ml_dtypes.bfloat16)
        in_maps.append(m)

    if _trace:
        _install_ntff_hook()
    nc = _get_compiled()
    res = run_bass_kernel_spmd(nc, in_maps, core_ids=list(range(B)),
                               trace=_trace)
    out = np.stack([np.asarray(res.results[b]["out"]).astype(np.float32)
                    .reshape(C, H, W) for b in range(B)])
    if _trace:
        return out, res
    return out


if __name__ == "__main__":
    rng = np.random.default_rng(0)
    inputs = {
        "x": rng.standard_normal((B, C, H, W), dtype=np.float32),
        "gamma": np.ones(C, dtype=np.float32),
        "beta": np.zeros(C, dtype=np.float32),
        "w_in": (rng.standard_normal((3 * HID, C), dtype=np.float32)
                 / np.sqrt(C)),
        "b_in": np.zeros(3 * HID, dtype=np.float32),
        "w_out": (rng.standard_normal((C, HID), dtype=np.float32)
                  / np.sqrt(HID)),
        "b_out": np.zeros(C, dtype=np.float32),
    }
    out = kernel(**inputs)
    print("kernel ran, out shape", out.shape)


# revision 17
# speedup vs baseline: 1.0338x; 1.0158x over previous
"""Trainium2 Bass kernel for nn_AttentionBlock (GroupNorm + 8-head attention
block on [8, 512, 32, 32], residual).

Sharding: pure data-parallel over batch B=8 across the 8 NeuronCores — one
batch element per core, weights replicated, zero collectives.

v5 = v4 (fp8e4 DoubleRow matmuls) + schedule/latency rework:
  - Head: x tiles stream in halves across both DMA queues before any other
    load; GN sums ride the idle ACT engine (Copy+accum), sum-of-squares on
    DVE, and the per-group algebra chain is DVE-resident (fewer cross-engine
    sem hops). Memsets issue after the loads.
  - Denominators: the den row is DMA'd straight out of the po PSUM (f32, no
    bf16 staging); reciprocals are scaled by 32 and broadcast per head as
    bf16 via the DRAM round trip (pairs 0-2) or a PE ones-matmul into a
    [128,1024] PSUM tile (pair 3, lowest latency).
  - attn: po rows are cast PSUM->fp8 with a 2^-5 prescale (unnormalized
    values reach ~733 > fp8 max), the odd head's rows hop partitions by DMA
    *before* the reciprocal arrives, and normalization happens in place
    (attn *= 32/den). This frees po banks at cast time and removes the au65
    staging tiles of v3/v4.
  - exp: odd-p sub1 tiles compute on the Pool engine via a Schraudolph
    bit-trick (uint8 = 1.4427*l + 32.46 IS the fp8e4m3 bit pattern of
    exp(l/8 - 2), ~3% rel err) so the ACT engine stops pacing the
    logits->exp->out2 pipeline. Attention carries ~7.6% of the output norm,
    so these approximations cost ~0.3% end-to-end (measured 6.3e-3 total,
    tolerance 2e-2).
  - proj_out: DoubleRow over chan-tile pairs, first-half accumulations run
    during the attention tail, output DMA'd as bf16.
"""
import sys

sys.path.insert(0, "/opt/trn_rl_repo")

import numpy as np
import ml_dtypes

import concourse.bass as bass
import concourse.bacc as bacc
import concourse.tile as tile
from concourse import mybir
from concourse.bass_utils import run_bass_kernel_spmd

F32 = mybir.dt.float32
BF16 = mybir.dt.bfloat16
FP8 = mybir.dt.float8e4
U8 = mybir.dt.uint8
ADD = mybir.AluOpType.add
MULT = mybir.AluOpType.mult
SUB = mybir.AluOpType.subtract
DR = mybir.MatmulPerfMode.DoubleRow

B, C, H, W = 8, 512, 32, 32
HW = H * W       # 1024
NG = 32          # groups
GS = C // NG     # 16 channels per group
NH = 8           # heads
HD = 64          # head dim
HID = NH * HD    # 512
NP = NH // 2     # 4 head pairs
EPS = 1e-6
SCALE = 1.0 / float(np.sqrt(HD))  # 0.125
EXP_SHIFT = -2.0  # exp(scale*l + shift): keeps e' under fp8e4 max (240)
ATT_PRE = 1.0 / 32  # prescale for the unnormalized po->fp8 cast
# Schraudolph fp8e4m3 bit-pattern exp: u8 = SCH_A*logit + SCH_B
SCH_A = 8.0 / np.log(2.0) * SCALE            # 1.44270
SCH_B = 8.0 * (np.log2(np.e) * EXP_SHIFT + 7.0) - 0.458
CT = C // 128    # 4 channel partition-tiles
PT = HW // 128   # 8 pixel partition-tiles
NA = PT // 2     # 4 kpix-tile pairs (DoubleRow accumulation steps)
GPT = NG // CT   # 8 groups per channel-tile
GN_INV = 1.0 / (GS * HW)          # 1/16384


def build_graph():
    nc = bacc.Bacc("TRN2", num_devices=8)

    x_ext = nc.declare_dram_parameter("x", [C, HW], BF16, isOutput=False)
    # fp8 pair-packed weights: [a][p, i, m] with contraction chan 128(2a+i)+p
    wqk_ext = nc.declare_dram_parameter("wqk8", [128, 2 * 2 * 1024], FP8,
                                        isOutput=False)
    wv_ext = nc.declare_dram_parameter("wv8", [128, 2 * 2 * 512], FP8,
                                       isOutput=False)
    wo_ext = nc.declare_dram_parameter("wo8", [128, 2 * 2 * 512], FP8,
                                       isOutput=False)
    # packed [128, 28] consts: 0:4 gamma, 4:8 beta, 8:16 b_in(q,k),
    # 16:20 b_out_eff, 20:28 gn_sel
    cpack_ext = nc.declare_dram_parameter("cpack", [128, 28], F32, isOutput=False)
    selT_ext = nc.declare_dram_parameter("gn_selT", [GPT + 2, 128], F32,
                                        isOutput=False)
    out_ext = nc.declare_dram_parameter("out", [C, HW], BF16, isOutput=True)

    rden_dram = nc.dram_tensor("rden_scratch", [NH, HW], BF16)

    with tile.TileContext(nc) as tc:
        with (
            tc.tile_pool(name="const", bufs=1) as const,
            tc.tile_pool(name="big", bufs=1) as big,
            tc.tile_pool(name="eT", bufs=1) as eTp,
            tc.tile_pool(name="small", bufs=2) as small,
        ):
            # ---------- loads: x0/x1 in halves across both queues first
            # (they gate the GN chain), then x2/x3, consts, weights ----------
            x_sb = [big.tile([128, HW], BF16, tag=f"x{t}", name=f"x{t}")
                    for t in range(CT)]
            nc.sync.dma_start(out=x_sb[0], in_=x_ext[0:128, :])
            nc.gpsimd.dma_start(out=x_sb[1], in_=x_ext[128:256, :])
            cpack_sb = const.tile([128, 28], F32)
            nc.gpsimd.dma_start(out=cpack_sb, in_=cpack_ext[:, :])
            selT_sb = const.tile([GPT, 128], F32)
            nc.gpsimd.dma_start(out=selT_sb, in_=selT_ext[0:GPT, :])
            nc.sync.dma_start(out=x_sb[2], in_=x_ext[256:384, :])
            nc.gpsimd.dma_start(out=x_sb[3], in_=x_ext[384:512, :])
            gamma_sb = cpack_sb[:, 0:4]
            beta_sb = cpack_sb[:, 4:8]
            b_in_sb = cpack_sb[:, 8:16]
            b_out_sb = cpack_sb[:, 16:20]
            sel_sb = cpack_sb[:, 20:28]
            # fp8 weight pair-tiles
            wqk_sb = [big.tile([128, 2, 2 * HID], FP8, tag=f"wqk{a}",
                               name=f"wqk{a}") for a in range(2)]
            for a in range(2):
                nc.sync.dma_start(
                    out=wqk_sb[a][:, :, :],
                    in_=wqk_ext[:, 2 * HID * 2 * a:2 * HID * 2 * (a + 1)]
                    .rearrange("p (i m) -> p i m", i=2))
            wv_sb = [big.tile([128, 2, HID], FP8, tag=f"wv{a}",
                              name=f"wv{a}") for a in range(2)]
            for a in range(2):
                nc.sync.dma_start(
                    out=wv_sb[a][:, :, :],
                    in_=wv_ext[:, HID * 2 * a:HID * 2 * (a + 1)]
                    .rearrange("p (i m) -> p i m", i=2))
            wo_sb = [big.tile([128, 2, HID], FP8, tag=f"wo{a}",
                              name=f"wo{a}") for a in range(2)]
            for a in range(2):
                nc.sync.dma_start(
                    out=wo_sb[a][:, :, :],
                    in_=wo_ext[:, HID * 2 * a:HID * 2 * (a + 1)]
                    .rearrange("p (i m) -> p i m", i=2))
            eshift_sb = const.tile([128, 1], F32)
            nc.vector.memset(eshift_sb, float(EXP_SHIFT))
            one_sb = const.tile([128, 1], F32)
            nc.vector.memset(one_sb, 1.0)
            # dummy ops hoist the ACT table loads (Sqrt/Identity and Exp
            # sets) into the idle pre-x window instead of the GN/exp path
            tl_scratch = small.tile([128, 1], F32, tag="tls", bufs=1)
            nc.scalar.activation(out=tl_scratch, in_=one_sb,
                                 func=mybir.ActivationFunctionType.Sqrt,
                                 bias=one_sb[:, :], scale=1.0)
            nc.scalar.activation(out=tl_scratch, in_=one_sb,
                                 func=mybir.ActivationFunctionType.Exp,
                                 scale=1.0, bias=one_sb[:, :])
            nc.scalar.activation(out=tl_scratch, in_=one_sb,
                                 func=mybir.ActivationFunctionType.Copy)

            # ---------- SBUF state ----------
            # h in fp8 pair-tiles: h_pair[a][:, i, :] = GN output chan-tile 2a+i
            h_pair = [big.tile([128, 2, HW], FP8, tag=f"h{a}", name=f"h{a}")
                      for a in range(2)]
            q_sb = [big.tile([128, HW], BF16, tag=f"q{m}", name=f"q{m}")
                    for m in range(NP)]
            k_sb = [big.tile([128, HW], BF16, tag=f"k{m}", name=f"k{m}")
                    for m in range(NP)]
            # vT pair-tiles: [a][p, i, head, c] = v for kpix 128(2a+i)+p,
            # c==HD is the denominator ones column; head-dim padded to HD+2
            # so the DoubleRow pair-stride stays 16B-aligned
            vT_pair = [big.tile([128, 2, NH, HD + 2], FP8, tag=f"vT{a}",
                                name=f"vT{a}") for a in range(NA)]
            # attn pair-tiles: [g][p, i, n] = attn chans 128(2g+i)+p
            attn_pair = [big.tile([128, 2, HW], FP8, tag=f"at{g}",
                                  name=f"at{g}") for g in range(2)]
            po_tiles = {}    # hp -> [po_sub0, po_sub1]
            eT_all = {}      # hp -> [[eT pair tiles sub0], [sub1]]

            def qk_burst(hp, which):
                """DoubleRow q or k projection for pair hp (4 matmuls)."""
                dest, off, bc = ((q_sb, 0, hp) if which == "q"
                                 else (k_sb, HID, 4 + hp))
                pp = pbig.tile([128, HW], F32, tag="pb",
                               name=f"{which}_acc{hp}")
                for n in range(2):
                    for a in range(2):
                        nc.tensor.matmul(
                            pp[:, 512 * n:512 * (n + 1)],
                            lhsT=wqk_sb[a][:, :, off + 128 * hp:
                                           off + 128 * (hp + 1)],
                            rhs=h_pair[a][:, :, 512 * n:512 * (n + 1)],
                            start=(a == 0), stop=(a == 1), perf_mode=DR)
                nc.vector.tensor_scalar(
                    out=dest[hp], in0=pp[:, :],
                    scalar1=b_in_sb[:, bc:bc + 1], scalar2=None, op0=ADD)

            def v_one(p):
                """One v kpix-tile (pv pool, own PSUM banks), DoubleRow."""
                pp = pv.tile([128, 512], F32, tag="pv")
                for a in range(2):
                    nc.tensor.matmul(
                        pp[:, :],
                        lhsT=h_pair[a][:, :, 128 * p:128 * (p + 1)],
                        rhs=wv_sb[a][:, :, :],
                        start=(a == 0), stop=(a == 1), perf_mode=DR)
                nc.vector.tensor_copy(
                    out=vT_pair[p // 2][:, p % 2, :, 0:HD],
                    in_=pp[:, :].rearrange("a (nh c) -> a nh c", nh=NH))

            def out2_step(hp, a, subs=(0, 1)):
                """One DoubleRow kpix-pair accumulation of pair hp's out2."""
                if a == 0 and hp not in po_tiles:
                    po_tiles[hp] = [
                        pop.tile([HD + 1, HW], F32, tag="po",
                                 name=f"po{2 * hp + s}") for s in range(2)]
                eTs = eT_all[hp]
                for sub in subs:
                    head = 2 * hp + sub
                    po_t = po_tiles[hp][sub]
                    for n in range(2):
                        nc.tensor.matmul(
                            po_t[:, 512 * n:512 * (n + 1)],
                            lhsT=vT_pair[a][:, :, head, 0:HD + 1],
                            rhs=eTs[sub][a][:, :, 512 * n:512 * (n + 1)],
                            start=(a == 0), stop=(a == NA - 1), perf_mode=DR)

            def emit_logits_exp(hp, out2_of=None, fillers=None,
                                o2_defer=()):
                """Logits (bf16) + exp->fp8 for pair hp; out2 DR steps of
                pair out2_of ride at odd p (when an eT pair completes).
                sub1 exps at odd p run on Pool via the Schraudolph trick."""
                eTs = [[eTp.tile([128, 2, HW], FP8, bufs=2, tag=f"eT{sub}_{a}",
                                 name=f"eT{hp}_{sub}_{a}") for a in range(NA)]
                       for sub in range(2)]
                eT_all[hp] = eTs
                o2_queue = []
                for p in range(PT):
                    pls = []
                    for sub in range(2):
                        lo = 64 * sub
                        pl = pbig.tile([128, HW], F32, tag="pb",
                                       name=f"pl{hp}_{sub}_{p}")
                        for n in range(2):
                            nc.tensor.matmul(
                                pl[:, 512 * n:512 * (n + 1)],
                                lhsT=k_sb[hp][lo:lo + 64, 128 * p:128 * (p + 1)],
                                rhs=q_sb[hp][lo:lo + 64, 512 * n:512 * (n + 1)],
                                start=True, stop=True)
                        pls.append(pl)
                    for sub in range(2):
                        dst = eTs[sub][p // 2][:, p % 2, :]
                        if sub == 1 and p % 2 == 1 and hp == 3:
                            # Schraudolph bit-trick exp on DVE: relieves the
                            # ACT engine in the out2(3) chase region
                            nc.vector.tensor_scalar(
                                out=dst.bitcast(U8), in0=pls[sub][:, :],
                                scalar1=float(SCH_A), scalar2=float(SCH_B),
                                op0=MULT, op1=ADD)
                        else:
                            nc.scalar.activation(
                                out=dst, in_=pls[sub][:, :],
                                func=mybir.ActivationFunctionType.Exp,
                                scale=SCALE, bias=eshift_sb[:, :])
                    if out2_of is not None and p % 2 == 1:
                        a = p // 2
                        if a in o2_defer:
                            o2_queue.append(a)
                        else:
                            for qq in o2_queue:
                                out2_step(out2_of, qq)
                            o2_queue.clear()
                            out2_step(out2_of, a)
                    for f in (fillers or {}).get(p, []):
                        f()

            pair_state = {}

            def finish_den(hp, pe_bcast=False):
                """po -> attn (fp8, prescaled 2^-5, unnormalized) + den
                reciprocal. The odd head's partition hop rides a DMA that
                doesn't wait for the reciprocal; normalization is done in
                place afterwards by finish_mul."""
                eT_all.pop(hp)
                pos = po_tiles.pop(hp)
                g, i = hp // 2, hp % 2
                den8 = small.tile([2, HW], FP8, tag="den8", bufs=2,
                                  name=f"den8_{hp}")
                den2 = small.tile([2, HW], F32, tag="den2", bufs=2,
                                  name=f"den2_{hp}")
                rr2 = small.tile([2, HW], F32, tag="rr2", bufs=2,
                                 name=f"rr2_{hp}")
                den_eng = nc.gpsimd if pe_bcast else nc.sync
                hop_eng = nc.sync if pe_bcast else nc.gpsimd
                # one prescaled fp8 cast per head covers attn rows AND the
                # denominator row; the 1/32 prescale cancels exactly in
                # attn = au8 * (1/(den/32))
                au8s = []
                for sub in range(2):
                    au8 = small.tile([HD + 1, HW], FP8, tag="attnu",
                                     bufs=4, name=f"attnu{2 * hp + sub}")
                    nc.vector.tensor_scalar_mul(au8, pos[sub][:, :],
                                                float(ATT_PRE))
                    den_eng.dma_start(out=den8[sub:sub + 1, :],
                                      in_=au8[HD:HD + 1, :])
                    au8s.append(au8)
                # both heads' rows hop into the attn tile unnormalized
                # (before rb exists); normalize is then ONE in-place mul
                hop_eng.dma_start(out=attn_pair[g][HD:128, i, :],
                                  in_=au8s[1][0:HD, :])
                den_eng.dma_start(out=attn_pair[g][0:HD, i, :],
                                  in_=au8s[0][0:HD, :])
                nc.vector.tensor_copy(out=den2, in_=den8)
                nc.vector.reciprocal_approx_fast(out=rr2, in_=den2)
                if pe_bcast:
                    # one matmul pair broadcasts BOTH heads: indicator rows
                    # of selT pick rr2 row 0 for partitions 0-63, row 1 for
                    # 64-127
                    bb = pop.tile([128, HW], F32, tag="po", name=f"bb{hp}")
                    for n in range(2):
                        nc.tensor.matmul(
                            bb[:, 512 * n:512 * (n + 1)],
                            lhsT=ind2_sb[:, :],
                            rhs=rr2[0:2, 512 * n:512 * (n + 1)],
                            start=True, stop=True)
                    pair_state[hp] = (au8s[0], bb)
                else:
                    rr2b = small.tile([2, HW], BF16, tag="rr2b", bufs=2,
                                      name=f"rr2b_{hp}")
                    nc.vector.tensor_copy(out=rr2b, in_=rr2)
                    for sub in range(2):
                        nc.sync.dma_start(
                            out=rden_dram[2 * hp + sub:2 * hp + sub + 1, :],
                            in_=rr2b[sub:sub + 1, :])
                    pair_state[hp] = (au8s[0], None)

            def finish_mul(hp):
                """One in-place normalize mul over both heads' rows."""
                au8e, bb = pair_state.pop(hp)
                g, i = hp // 2, hp % 2
                if bb is None:
                    rbt = small.tile([128, HW], BF16, tag="rb", bufs=2,
                                     name=f"rb{hp}")
                    for sub in range(2):
                        bcast_ap = bass.AP(
                            tensor=rden_dram[:, :].tensor,
                            offset=(2 * hp + sub) * HW,
                            ap=[[0, HD], [1, HW]])
                        nc.sync.dma_start(out=rbt[64 * sub:64 * (sub + 1), :],
                                          in_=bcast_ap)
                    rb = rbt[:, :]
                    nc.gpsimd.tensor_mul(attn_pair[g][:, i, :],
                                         attn_pair[g][:, i, :], rb)
                else:
                    nc.vector.tensor_mul(attn_pair[g][:, i, :],
                                         attn_pair[g][:, i, :], bb[:, :])

            with tc.tile_pool(name="pbig", bufs=2, space="PSUM") as pbig:
                # GN per tile + qk0 DR accumulation rides along so the first
                # exp fires as soon as x1 lands.
                ppq0 = pbig.tile([128, HW], F32, tag="pb", name="q_acc0")
                ppk0 = pbig.tile([128, HW], F32, tag="pb", name="k_acc0")
                with tc.tile_pool(name="ps_gn", bufs=2, space="PSUM") as ps_gn:
                    eps_sb = small.tile([GPT, 1], F32, tag="eps_c", bufs=1)
                    nc.gpsimd.memset(eps_sb, float(EPS))
                    sq_scratch = small.tile([128, HW], BF16, tag="sqs",
                                            bufs=1)
                    sts = []

                    cp_scratch = small.tile([128, HW], BF16, tag="cps",
                                            bufs=1)

                    def gn_stats(t):
                        st = small.tile([128, 2], F32, tag=f"st{t}", bufs=1,
                                        name=f"st{t}")
                        sts.append(st)
                        nc.scalar.activation(
                            out=cp_scratch, in_=x_sb[t][:, :],
                            func=mybir.ActivationFunctionType.Copy,
                            accum_out=st[:, 0:1])
                        nc.vector.scalar_tensor_tensor(
                            out=sq_scratch, in0=x_sb[t][:, :], scalar=1.0,
                            in1=x_sb[t][:, :],
                            op0=mybir.AluOpType.bypass, op1=MULT,
                            accum_out=st[:, 1:2])

                    def gn_tile(t):
                        st = sts[t]
                        gpsum = ps_gn.tile([GPT, 2], F32, tag="gps")
                        nc.tensor.matmul(gpsum[:, :], lhsT=sel_sb[:, :],
                                         rhs=st[:, :], start=True, stop=True)
                        # grp cols: 0 rstd, 1 mean*rstd, 2 mean, 3 E[x^2]
                        grp = small.tile([GPT, 4], F32, tag="grp", bufs=2,
                                         name=f"grp{t}")
                        nc.vector.tensor_scalar_mul(grp[:, 2:4],
                                                    gpsum[:, 0:2], GN_INV)
                        nc.vector.tensor_mul(grp[:, 0:1], grp[:, 2:3],
                                             grp[:, 2:3])
                        nc.vector.tensor_sub(grp[:, 0:1], grp[:, 3:4],
                                             grp[:, 0:1])
                        nc.scalar.activation(
                            out=grp[:, 0:1], in_=grp[:, 0:1],
                            func=mybir.ActivationFunctionType.Sqrt,
                            bias=eps_sb[:, :], scale=1.0)
                        nc.vector.reciprocal(out=grp[:, 0:1], in_=grp[:, 0:1])
                        nc.vector.tensor_mul(grp[:, 1:2], grp[:, 2:3],
                                             grp[:, 0:1])
                        epsum = ps_gn.tile([128, 2], F32, tag="eps")
                        nc.tensor.matmul(epsum[:, :], lhsT=selT_sb[:, :],
                                         rhs=grp[:, 0:2], start=True,
                                         stop=True)
                        ga = small.tile([128, 1], F32, tag=f"ga{t}", bufs=1,
                                        name=f"ga{t}")
                        gd = small.tile([128, 1], F32, tag=f"gd{t}", bufs=1,
                                        name=f"gd{t}")
                        nc.vector.tensor_mul(ga[:, :], gamma_sb[:, t:t + 1],
                                             epsum[:, 0:1])
                        nc.vector.tensor_mul(gd[:, :], gamma_sb[:, t:t + 1],
                                             epsum[:, 1:2])
                        nc.vector.tensor_sub(gd[:, :], beta_sb[:, t:t + 1],
                                             gd[:, :])
                        nc.vector.tensor_scalar(
                            out=h_pair[t // 2][:, t % 2, :],
                            in0=x_sb[t][:, :],
                            scalar1=ga[:, :], scalar2=gd[:, :],
                            op0=MULT, op1=ADD)

                    # stats(t) immediately before tile t's chain: the
                    # gpsum matmul then fires as soon as its own stats land
                    for t in range(CT):
                        gn_stats(t)
                        gn_tile(t)
                        if t % 2 == 1:
                            a = t // 2
                            for n in range(2):
                                nc.tensor.matmul(
                                    ppq0[:, 512 * n:512 * (n + 1)],
                                    lhsT=wqk_sb[a][:, :, 0:128],
                                    rhs=h_pair[a][:, :, 512 * n:512 * (n + 1)],
                                    start=(a == 0), stop=(a == 1),
                                    perf_mode=DR)
                                nc.tensor.matmul(
                                    ppk0[:, 512 * n:512 * (n + 1)],
                                    lhsT=wqk_sb[a][:, :, HID:HID + 128],
                                    rhs=h_pair[a][:, :, 512 * n:512 * (n + 1)],
                                    start=(a == 0), stop=(a == 1),
                                    perf_mode=DR)
                nc.vector.tensor_scalar(
                    out=q_sb[0], in0=ppq0[:, :],
                    scalar1=b_in_sb[:, 0:1], scalar2=None, op0=ADD)
                # k0 evict on ACT (idle pre-exp) so it runs parallel to the
                # q0 evict on DVE — both gate the first logits matmul.
                nc.scalar.activation(
                    out=k_sb[0], in_=ppk0[:, :],
                    func=mybir.ActivationFunctionType.Identity,
                    bias=b_in_sb[:, 4:5], scale=1.0)
                for a in range(NA):
                    for i in range(2):
                        nc.vector.memset(vT_pair[a][:, i, :, HD:HD + 1], 1.0)
                ind2_sb = const.tile([2, 128], F32)
                nc.sync.dma_start(out=ind2_sb,
                                  in_=selT_ext[GPT:GPT + 2, :])
                with tc.tile_pool(name="pv", bufs=2, space="PSUM") as pv:
                    emit_logits_exp(0, fillers={
                        0: [lambda: v_one(0)],
                        1: [lambda: v_one(1)],
                        2: [lambda: qk_burst(1, "q")],
                        3: [lambda: v_one(2)],
                        4: [lambda: v_one(3)],
                        5: [lambda: qk_burst(1, "k")],
                        6: [lambda: v_one(4), lambda: v_one(5)],
                        7: [lambda: v_one(6), lambda: v_one(7)],
                    })
                with tc.tile_pool(name="po", bufs=2, space="PSUM") as pop:
                    emit_logits_exp(1, out2_of=0, o2_defer=(1,), fillers={
                        2: [lambda: qk_burst(2, "q")],
                        5: [lambda: qk_burst(2, "k")],
                    })
                    finish_den(0)
                    emit_logits_exp(2, out2_of=1, o2_defer=(1,), fillers={
                        2: [lambda: qk_burst(3, "q")],
                        5: [lambda: qk_burst(3, "k")],
                    })
                    finish_den(1)
                    finish_mul(0)
                    emit_logits_exp(3, out2_of=2)
                    finish_den(2)
                    finish_mul(1)
                    # chase pair 3 sub-major: sub0's po finishes (and its
                    # au cast + den extract start) while sub1 still matmuls
                    for a in range(NA):
                        out2_step(3, a, subs=(0,))
                    for a in range(NA):
                        out2_step(3, a, subs=(1,))
                    finish_den(3, pe_bcast=True)
                    finish_mul(2)
                    finish_mul(3)

            # ---------- proj_out + bias + residual ----------
            # DoubleRow over chan-tile pairs: 2 accumulation steps per chunk.
            # pre = step a=0 (runs as soon as attn_pair[0] lands), fin = a=1
            # + evict + bf16 DMA out.
            with tc.tile_pool(name="ps_pout", bufs=8, space="PSUM") as ps_pout:
                # ring of 8: every chunk's first DoubleRow accumulation can
                # run during the attention tail; after mul(3) only the 8
                # closing matmuls + evicts remain
                def pout_chunk_pre(m, n, pps):
                    pp = ps_pout.tile([128, 512], F32, tag="pp",
                                      name=f"po_{m}_{n}")
                    pps[(m, n)] = pp
                    nc.tensor.matmul(
                        pp[:, :],
                        lhsT=wo_sb[0][:, :, 128 * m:128 * (m + 1)],
                        rhs=attn_pair[0][:, :, 512 * n:512 * (n + 1)],
                        start=True, stop=False, perf_mode=DR)

                def pout_chunk_fin(m, n, pps):
                    pp = pps.pop((m, n))
                    nc.tensor.matmul(
                        pp[:, :],
                        lhsT=wo_sb[1][:, :, 128 * m:128 * (m + 1)],
                        rhs=attn_pair[1][:, :, 512 * n:512 * (n + 1)],
                        start=False, stop=True, perf_mode=DR)
                    o_sb = small.tile([128, 512], BF16, tag="osb", bufs=4)
                    nc.vector.scalar_tensor_tensor(
                        out=o_sb, in0=pp[:, :],
                        scalar=b_out_sb[:, m:m + 1],
                        in1=x_sb[m][:, 512 * n:512 * (n + 1)],
                        op0=ADD, op1=ADD)
                    for hh in range(2):
                        deng = nc.gpsimd if (2 * m + n + hh) % 2 else nc.sync
                        deng.dma_start(
                            out=out_ext[128 * m:128 * (m + 1),
                                        512 * n + 256 * hh:
                                        512 * n + 256 * (hh + 1)],
                            in_=o_sb[:, 256 * hh:256 * (hh + 1)])

                pps = {}
                chunks = [(m, n) for n in range(2) for m in range(CT)]
                for m, n in chunks:
                    pout_chunk_pre(m, n, pps)
                for m, n in chunks:
                    pout_chunk_fin(m, n, pps)
    return nc


def _install_ntff_hook():
    """The agent image's antenv lacks axon_hooks; synthesize it so
    run_bass_kernel_spmd(trace=True) can reach the NTFF profiler."""
    import types
    if "antenv.axon_hooks" in sys.modules:
        return
    mod = types.ModuleType("antenv.axon_hooks")
    mod._hook = None

    def set_axon_ntff_profile_hook(hook):
        mod._hook = hook

    def get_axon_ntff_profile_hook():
        return mod._hook

    mod.set_axon_ntff_profile_hook = set_axon_ntff_profile_hook
    mod.get_axon_ntff_profile_hook = get_axon_ntff_profile_hook
    sys.modules["antenv.axon_hooks"] = mod
    try:
        from trn_agent_boot.trn_boot import _ntff_profile_via_ctypes
        hook = _ntff_profile_via_ctypes("/opt/axon/libaxon_pjrt.so")
        if hook is not None:
            set_axon_ntff_profile_hook(hook)
    except Exception as e:  # degrade to no tracing
        print("ntff hook setup failed:", e)


_COMPILED = None


def _get_compiled():
    global _COMPILED
    if _COMPILED is None:
        nc = build_graph()
        nc.compile()
        _COMPILED = nc
    return _COMPILED


def _make_consts():
    # within any 128-channel tile, partition p belongs to local group p//16;
    # rows GPT/GPT+1 are even/odd-head indicator rows for the den broadcast
    sel = np.zeros((128, GPT), dtype=np.float32)
    selT = np.zeros((GPT + 2, 128), dtype=np.float32)
    for p in range(128):
        sel[p, p // GS] = 1.0
        selT[p // GS, p] = 1.0
    selT[GPT, 0:64] = 1.0
    selT[GPT + 1, 64:128] = 1.0
    return sel, selT


def _pm(v, cols):
    """[cols*128] vector -> partition-major [128, cols]."""
    return np.ascontiguousarray(v.reshape(cols, 128).T)


def _pack_pairs(w, m_dim):
    """w [m_dim, 512] -> [128, 2*2*m_dim] fp8: flat[a] [p, i, m] =
    w[m, 128*(2a+i)+p]."""
    wT = w.T.reshape(2, 2, 128, m_dim)           # [a, i, p, m]
    out = np.transpose(wT, (0, 2, 1, 3))          # [a, p, i, m]
    out = out.reshape(2, 128, 2 * m_dim)          # [a, p, i*m]
    out = np.concatenate([out[0], out[1]], axis=1)  # [p, a*i*m]
    return np.ascontiguousarray(out).astype(ml_dtypes.float8_e4m3)


def kernel(x, gamma, beta, w_in, b_in, w_out, b_out, _trace=False):
    x = np.asarray(x, dtype=np.float32)
    gamma = np.asarray(gamma, dtype=np.float32)
    beta = np.asarray(beta, dtype=np.float32)
    w_in = np.asarray(w_in, dtype=np.float32)
    b_in = np.asarray(b_in, dtype=np.float32)
    w_out = np.asarray(w_out, dtype=np.float32)
    b_out = np.asarray(b_out, dtype=np.float32)

    wqk8 = _pack_pairs(w_in[0:2 * HID], 2 * HID)   # q rows 0:512, k 512:1024
    wv8 = _pack_pairs(w_in[2 * HID:3 * HID], HID)
    wo8 = _pack_pairs(w_out, HID)
    sel, selT = _make_consts()
    # fold v-bias through proj_out: softmax rows sum to 1, so the attention
    # output is attn_raw + b_v exactly; w_out @ b_v + b_out replaces b_out.
    b_v = b_in[2 * HID:3 * HID]
    b_out_eff = b_out + w_out.astype(np.float64) @ b_v.astype(np.float64)
    b_out_eff = b_out_eff.astype(np.float32)
    cpack = np.zeros((128, 28), dtype=np.float32)
    cpack[:, 0:4] = _pm(gamma, CT)
    cpack[:, 4:8] = _pm(beta, CT)
    cpack[:, 8:16] = _pm(b_in[0:2 * HID], 8)
    cpack[:, 16:20] = _pm(b_out_eff, CT)
    cpack[:, 20:28] = sel
    common = {
        "wqk8": wqk8,
        "wv8": wv8,
        "wo8": wo8,
        "cpack": cpack,
        "gn_selT": selT,
    }
    in_maps = []
    for b in range(B):
        m = dict(common)
        m["x"] = np.ascontiguousarray(x[b].reshape(C, HW)).astype(
            ml_dtypes.bfloat16)
        in_maps.append(m)

    if _trace:
        _install_ntff_hook()
    nc = _get_compiled()
    res = run_bass_kernel_spmd(nc, in_maps, core_ids=list(range(B)),
                               trace=_trace)
    out = np.stack([np.asarray(res.results[b]["out"]).astype(np.float32)
                    .reshape(C, H, W) for b in range(B)])
    if _trace:
        return out, res
    return out


if __name__ == "__main__":
    rng = np.random.default_rng(0)
    inputs = {
        "x": rng.standard_normal((B, C, H, W), dtype=np.float32),
        "gamma": np.ones(C, dtype=np.float32),
        "beta": np.zeros(C, dtype=np.float32),
        "w_in": (rng.standard_normal((3 * HID, C), dtype=np.float32)
                 / np.sqrt(C)),
        "b_in": np.zeros(3 * HID, dtype=np.float32),
        "w_out": (rng.standard_normal((C, HID), dtype=np.float32)
                  / np.sqrt(HID)),
        "b_out": np.zeros(C, dtype=np.float32),
    }
    out = kernel(**inputs)
    print("kernel ran, out shape", out.shape)


# revision 18
# speedup vs baseline: 1.0610x; 1.0263x over previous
"""Trainium2 Bass kernel for nn_AttentionBlock (GroupNorm + 8-head attention
block on [8, 512, 32, 32], residual).

Sharding: pure data-parallel over batch B=8 across the 8 NeuronCores — one
batch element per core, weights replicated, zero collectives.

v5 = v4 (fp8e4 DoubleRow matmuls) + schedule/latency rework:
  - Head: x tiles stream in halves across both DMA queues before any other
    load; GN sums ride the idle ACT engine (Copy+accum), sum-of-squares on
    DVE, and the per-group algebra chain is DVE-resident (fewer cross-engine
    sem hops). Memsets issue after the loads.
  - Denominators: the den row is DMA'd straight out of the po PSUM (f32, no
    bf16 staging); reciprocals are scaled by 32 and broadcast per head as
    bf16 via the DRAM round trip (pairs 0-2) or a PE ones-matmul into a
    [128,1024] PSUM tile (pair 3, lowest latency).
  - attn: po rows are cast PSUM->fp8 with a 2^-5 prescale (unnormalized
    values reach ~733 > fp8 max), the odd head's rows hop partitions by DMA
    *before* the reciprocal arrives, and normalization happens in place
    (attn *= 32/den). This frees po banks at cast time and removes the au65
    staging tiles of v3/v4.
  - exp: odd-p sub1 tiles compute on the Pool engine via a Schraudolph
    bit-trick (uint8 = 1.4427*l + 32.46 IS the fp8e4m3 bit pattern of
    exp(l/8 - 2), ~3% rel err) so the ACT engine stops pacing the
    logits->exp->out2 pipeline. Attention carries ~7.6% of the output norm,
    so these approximations cost ~0.3% end-to-end (measured 6.3e-3 total,
    tolerance 2e-2).
  - proj_out: DoubleRow over chan-tile pairs, first-half accumulations run
    during the attention tail, output DMA'd as bf16.
"""
import sys

sys.path.insert(0, "/opt/trn_rl_repo")

import numpy as np
import ml_dtypes

import concourse.bass as bass
import concourse.bacc as bacc
import concourse.tile as tile
from concourse import mybir
from concourse.bass_utils import run_bass_kernel_spmd

F32 = mybir.dt.float32
BF16 = mybir.dt.bfloat16
FP8 = mybir.dt.float8e4
U8 = mybir.dt.uint8
ADD = mybir.AluOpType.add
MULT = mybir.AluOpType.mult
SUB = mybir.AluOpType.subtract
DR = mybir.MatmulPerfMode.DoubleRow

B, C, H, W = 8, 512, 32, 32
HW = H * W       # 1024
NG = 32          # groups
GS = C // NG     # 16 channels per group
NH = 8           # heads
HD = 64          # head dim
HID = NH * HD    # 512
NP = NH // 2     # 4 head pairs
EPS = 1e-6
SCALE = 1.0 / float(np.sqrt(HD))  # 0.125
EXP_SHIFT = -2.0  # exp(scale*l + shift): keeps e' under fp8e4 max (240)
ATT_PRE = 1.0 / 32  # prescale for the unnormalized po->fp8 cast
# Schraudolph fp8e4m3 bit-pattern exp: u8 = SCH_A*logit + SCH_B
SCH_A = 8.0 / np.log(2.0) * SCALE            # 1.44270
SCH_B = 8.0 * (np.log2(np.e) * EXP_SHIFT + 7.0) - 0.458
CT = C // 128    # 4 channel partition-tiles
PT = HW // 128   # 8 pixel partition-tiles
NA = PT // 2     # 4 kpix-tile pairs (DoubleRow accumulation steps)
GPT = NG // CT   # 8 groups per channel-tile
GN_INV = 1.0 / (GS * HW)          # 1/16384


def build_graph():
    nc = bacc.Bacc("TRN2", num_devices=8)

    x_ext = nc.declare_dram_parameter("x", [C, HW], BF16, isOutput=False)
    # fp8 pair-packed weights: [a][p, i, m] with contraction chan 128(2a+i)+p
    wqk_ext = nc.declare_dram_parameter("wqk8", [128, 2 * 2 * 1024], FP8,
                                        isOutput=False)
    wv_ext = nc.declare_dram_parameter("wv8", [128, 2 * 2 * 512], FP8,
                                       isOutput=False)
    wo_ext = nc.declare_dram_parameter("wo8", [128, 2 * 2 * 512], FP8,
                                       isOutput=False)
    # packed [128, 28] consts: 0:4 gamma, 4:8 beta, 8:16 b_in(q,k),
    # 16:20 b_out_eff, 20:28 gn_sel
    cpack_ext = nc.declare_dram_parameter("cpack", [128, 28], F32, isOutput=False)
    selT_ext = nc.declare_dram_parameter("gn_selT", [GPT + 2, 128], F32,
                                        isOutput=False)
    out_ext = nc.declare_dram_parameter("out", [C, HW], BF16, isOutput=True)

    rden_dram = nc.dram_tensor("rden_scratch", [NH, HW], BF16)

    with tile.TileContext(nc) as tc:
        with (
            tc.tile_pool(name="const", bufs=1) as const,
            tc.tile_pool(name="big", bufs=1) as big,
            tc.tile_pool(name="eT", bufs=1) as eTp,
            tc.tile_pool(name="small", bufs=2) as small,
        ):
            # ---------- loads: x0/x1 in halves across both queues first
            # (they gate the GN chain), then x2/x3, consts, weights ----------
            x_sb = [big.tile([128, HW], BF16, tag=f"x{t}", name=f"x{t}")
                    for t in range(CT)]
            nc.sync.dma_start(out=x_sb[0], in_=x_ext[0:128, :])
            nc.gpsimd.dma_start(out=x_sb[1], in_=x_ext[128:256, :])
            cpack_sb = const.tile([128, 28], F32)
            nc.gpsimd.dma_start(out=cpack_sb, in_=cpack_ext[:, :])
            selT_sb = const.tile([GPT, 128], F32)
            nc.gpsimd.dma_start(out=selT_sb, in_=selT_ext[0:GPT, :])
            nc.sync.dma_start(out=x_sb[2], in_=x_ext[256:384, :])
            nc.gpsimd.dma_start(out=x_sb[3], in_=x_ext[384:512, :])
            gamma_sb = cpack_sb[:, 0:4]
            beta_sb = cpack_sb[:, 4:8]
            b_in_sb = cpack_sb[:, 8:16]
            b_out_sb = cpack_sb[:, 16:20]
            sel_sb = cpack_sb[:, 20:28]
            # fp8 weight pair-tiles
            wqk_sb = [big.tile([128, 2, 2 * HID], FP8, tag=f"wqk{a}",
                               name=f"wqk{a}") for a in range(2)]
            for a in range(2):
                nc.sync.dma_start(
                    out=wqk_sb[a][:, :, :],
                    in_=wqk_ext[:, 2 * HID * 2 * a:2 * HID * 2 * (a + 1)]
                    .rearrange("p (i m) -> p i m", i=2))
            wv_sb = [big.tile([128, 2, HID], FP8, tag=f"wv{a}",
                              name=f"wv{a}") for a in range(2)]
            for a in range(2):
                nc.sync.dma_start(
                    out=wv_sb[a][:, :, :],
                    in_=wv_ext[:, HID * 2 * a:HID * 2 * (a + 1)]
                    .rearrange("p (i m) -> p i m", i=2))
            wo_sb = [big.tile([128, 2, HID], FP8, tag=f"wo{a}",
                              name=f"wo{a}") for a in range(2)]
            for a in range(2):
                nc.sync.dma_start(
                    out=wo_sb[a][:, :, :],
                    in_=wo_ext[:, HID * 2 * a:HID * 2 * (a + 1)]
                    .rearrange("p (i m) -> p i m", i=2))
            eshift_sb = const.tile([128, 1], F32)
            nc.vector.memset(eshift_sb, float(EXP_SHIFT))
            one_sb = const.tile([128, 1], F32)
            nc.vector.memset(one_sb, 1.0)
            # dummy ops hoist the ACT table loads (Sqrt/Identity and Exp
            # sets) into the idle pre-x window instead of the GN/exp path
            tl_scratch = small.tile([128, 1], F32, tag="tls", bufs=1)
            nc.scalar.activation(out=tl_scratch, in_=one_sb,
                                 func=mybir.ActivationFunctionType.Sqrt,
                                 bias=one_sb[:, :], scale=1.0)
            nc.scalar.activation(out=tl_scratch, in_=one_sb,
                                 func=mybir.ActivationFunctionType.Exp,
                                 scale=1.0, bias=one_sb[:, :])
            nc.scalar.activation(out=tl_scratch, in_=one_sb,
                                 func=mybir.ActivationFunctionType.Copy)

            # ---------- SBUF state ----------
            # h in fp8 pair-tiles: h_pair[a][:, i, :] = GN output chan-tile 2a+i
            h_pair = [big.tile([128, 2, HW], FP8, tag=f"h{a}", name=f"h{a}")
                      for a in range(2)]
            q_sb = [big.tile([128, HW], BF16, tag=f"q{m}", name=f"q{m}")
                    for m in range(NP)]
            k_sb = [big.tile([128, HW], BF16, tag=f"k{m}", name=f"k{m}")
                    for m in range(NP)]
            # vT pair-tiles: [a][p, i, head, c] = v for kpix 128(2a+i)+p,
            # c==HD is the denominator ones column; head-dim padded to HD+2
            # so the DoubleRow pair-stride stays 16B-aligned
            vT_pair = [big.tile([128, 2, NH, HD + 2], FP8, tag=f"vT{a}",
                                name=f"vT{a}") for a in range(NA)]
            # attn pair-tiles: [g][p, i, n] = attn chans 128(2g+i)+p
            attn_pair = [big.tile([128, 2, HW], FP8, tag=f"at{g}",
                                  name=f"at{g}") for g in range(2)]
            po_tiles = {}    # hp -> [po_sub0, po_sub1]
            eT_all = {}      # hp -> [[eT pair tiles sub0], [sub1]]

            def qk_burst(hp, which):
                """DoubleRow q or k projection for pair hp (4 matmuls)."""
                dest, off, bc = ((q_sb, 0, hp) if which == "q"
                                 else (k_sb, HID, 4 + hp))
                pp = pbig.tile([128, HW], F32, tag="pb",
                               name=f"{which}_acc{hp}")
                for n in range(2):
                    for a in range(2):
                        nc.tensor.matmul(
                            pp[:, 512 * n:512 * (n + 1)],
                            lhsT=wqk_sb[a][:, :, off + 128 * hp:
                                           off + 128 * (hp + 1)],
                            rhs=h_pair[a][:, :, 512 * n:512 * (n + 1)],
                            start=(a == 0), stop=(a == 1), perf_mode=DR)
                nc.vector.tensor_scalar(
                    out=dest[hp], in0=pp[:, :],
                    scalar1=b_in_sb[:, bc:bc + 1], scalar2=None, op0=ADD)

            def v_one(p):
                """One v kpix-tile (pv pool, own PSUM banks), DoubleRow."""
                pp = pv.tile([128, 512], F32, tag="pv")
                for a in range(2):
                    nc.tensor.matmul(
                        pp[:, :],
                        lhsT=h_pair[a][:, :, 128 * p:128 * (p + 1)],
                        rhs=wv_sb[a][:, :, :],
                        start=(a == 0), stop=(a == 1), perf_mode=DR)
                nc.vector.tensor_copy(
                    out=vT_pair[p // 2][:, p % 2, :, 0:HD],
                    in_=pp[:, :].rearrange("a (nh c) -> a nh c", nh=NH))

            def out2_step(hp, a, subs=(0, 1)):
                """One DoubleRow kpix-pair accumulation of pair hp's out2."""
                if a == 0 and hp not in po_tiles:
                    po_tiles[hp] = [
                        pop.tile([HD + 1, HW], F32, tag="po",
                                 name=f"po{2 * hp + s}") for s in range(2)]
                eTs = eT_all[hp]
                for sub in subs:
                    head = 2 * hp + sub
                    po_t = po_tiles[hp][sub]
                    for n in range(2):
                        nc.tensor.matmul(
                            po_t[:, 512 * n:512 * (n + 1)],
                            lhsT=vT_pair[a][:, :, head, 0:HD + 1],
                            rhs=eTs[sub][a][:, :, 512 * n:512 * (n + 1)],
                            start=(a == 0), stop=(a == NA - 1), perf_mode=DR)

            def emit_logits_exp(hp, out2_of=None, fillers=None,
                                o2_defer=()):
                """Logits (bf16) + exp->fp8 for pair hp; out2 DR steps of
                pair out2_of ride at odd p (when an eT pair completes).
                sub1 exps at odd p run on Pool via the Schraudolph trick."""
                eTs = [[eTp.tile([128, 2, HW], FP8, bufs=2, tag=f"eT{sub}_{a}",
                                 name=f"eT{hp}_{sub}_{a}") for a in range(NA)]
                       for sub in range(2)]
                eT_all[hp] = eTs
                o2_queue = []
                for p in range(PT):
                    pls = []
                    for sub in range(2):
                        lo = 64 * sub
                        pl = pbig.tile([128, HW], F32, tag="pb",
                                       name=f"pl{hp}_{sub}_{p}")
                        for n in range(2):
                            nc.tensor.matmul(
                                pl[:, 512 * n:512 * (n + 1)],
                                lhsT=k_sb[hp][lo:lo + 64, 128 * p:128 * (p + 1)],
                                rhs=q_sb[hp][lo:lo + 64, 512 * n:512 * (n + 1)],
                                start=True, stop=True)
                        pls.append(pl)
                    for sub in range(2):
                        dst = eTs[sub][p // 2][:, p % 2, :]
                        if sub == 1 and p % 2 == 1 and hp == 3:
                            # Schraudolph bit-trick exp on DVE: relieves the
                            # ACT engine in the out2(3) chase region
                            nc.vector.tensor_scalar(
                                out=dst.bitcast(U8), in0=pls[sub][:, :],
                                scalar1=float(SCH_A), scalar2=float(SCH_B),
                                op0=MULT, op1=ADD)
                        else:
                            nc.scalar.activation(
                                out=dst, in_=pls[sub][:, :],
                                func=mybir.ActivationFunctionType.Exp,
                                scale=SCALE, bias=eshift_sb[:, :])
                    if out2_of is not None and p % 2 == 1:
                        a = p // 2
                        if a in o2_defer:
                            o2_queue.append(a)
                        else:
                            for qq in o2_queue:
                                out2_step(out2_of, qq)
                            o2_queue.clear()
                            out2_step(out2_of, a)
                    for f in (fillers or {}).get(p, []):
                        f()

            pair_state = {}

            def finish_den(hp, pe_bcast=False):
                """po -> attn (fp8, prescaled 2^-5, unnormalized) + den
                reciprocal. The odd head's partition hop rides a DMA that
                doesn't wait for the reciprocal; normalization is done in
                place afterwards by finish_mul."""
                eT_all.pop(hp)
                pos = po_tiles.pop(hp)
                g, i = hp // 2, hp % 2
                den8 = small.tile([2, HW], FP8, tag="den8", bufs=2,
                                  name=f"den8_{hp}")
                den2 = small.tile([2, HW], F32, tag="den2", bufs=2,
                                  name=f"den2_{hp}")
                rr2 = small.tile([2, HW], F32, tag="rr2", bufs=2,
                                 name=f"rr2_{hp}")
                den_eng = nc.gpsimd if pe_bcast else nc.sync
                hop_eng = nc.sync if pe_bcast else nc.gpsimd
                # one prescaled fp8 cast per head covers attn rows AND the
                # denominator row; the 1/32 prescale cancels exactly in
                # attn = au8 * (1/(den/32))
                au8s = []
                for sub in range(2):
                    au8 = small.tile([HD + 1, HW], FP8, tag="attnu",
                                     bufs=4, name=f"attnu{2 * hp + sub}")
                    nc.vector.tensor_scalar_mul(au8, pos[sub][:, :],
                                                float(ATT_PRE))
                    den_eng.dma_start(out=den8[sub:sub + 1, :],
                                      in_=au8[HD:HD + 1, :])
                    au8s.append(au8)
                # odd head's rows hop into the attn tile unnormalized
                # (before rb exists); for the tail pair the even head hops
                # too so normalize collapses to ONE in-place mul
                hop_eng.dma_start(out=attn_pair[g][HD:128, i, :],
                                  in_=au8s[1][0:HD, :])
                if pe_bcast:
                    den_eng.dma_start(out=attn_pair[g][0:HD, i, :],
                                      in_=au8s[0][0:HD, :])
                nc.vector.tensor_copy(out=den2, in_=den8)
                nc.vector.reciprocal_approx_fast(out=rr2, in_=den2)
                if pe_bcast:
                    # one matmul pair broadcasts BOTH heads: indicator rows
                    # of selT pick rr2 row 0 for partitions 0-63, row 1 for
                    # 64-127
                    bb = pop.tile([128, HW], F32, tag="po", name=f"bb{hp}")
                    for n in range(2):
                        nc.tensor.matmul(
                            bb[:, 512 * n:512 * (n + 1)],
                            lhsT=ind2_sb[:, :],
                            rhs=rr2[0:2, 512 * n:512 * (n + 1)],
                            start=True, stop=True)
                    pair_state[hp] = (au8s[0], bb)
                else:
                    rr2b = small.tile([2, HW], BF16, tag="rr2b", bufs=2,
                                      name=f"rr2b_{hp}")
                    nc.vector.tensor_copy(out=rr2b, in_=rr2)
                    for sub in range(2):
                        nc.sync.dma_start(
                            out=rden_dram[2 * hp + sub:2 * hp + sub + 1, :],
                            in_=rr2b[sub:sub + 1, :])
                    pair_state[hp] = (au8s[0], None)

            def finish_mul(hp):
                """One in-place normalize mul over both heads' rows."""
                au8e, bb = pair_state.pop(hp)
                g, i = hp // 2, hp % 2
                if bb is None:
                    rbt = small.tile([128, HW], BF16, tag="rb", bufs=2,
                                     name=f"rb{hp}")
                    for sub in range(2):
                        bcast_ap = bass.AP(
                            tensor=rden_dram[:, :].tensor,
                            offset=(2 * hp + sub) * HW,
                            ap=[[0, HD], [1, HW]])
                        nc.sync.dma_start(out=rbt[64 * sub:64 * (sub + 1), :],
                                          in_=bcast_ap)
                    nc.gpsimd.tensor_mul(attn_pair[g][0:HD, i, :],
                                         au8e[0:HD, :], rbt[0:HD, :])
                    nc.gpsimd.tensor_mul(attn_pair[g][HD:128, i, :],
                                         attn_pair[g][HD:128, i, :],
                                         rbt[HD:128, :])
                else:
                    nc.vector.tensor_mul(attn_pair[g][:, i, :],
                                         attn_pair[g][:, i, :], bb[:, :])

            with tc.tile_pool(name="pbig", bufs=2, space="PSUM") as pbig:
                # GN per tile + qk0 DR accumulation rides along so the first
                # exp fires as soon as x1 lands.
                ppq0 = pbig.tile([128, HW], F32, tag="pb", name="q_acc0")
                ppk0 = pbig.tile([128, HW], F32, tag="pb", name="k_acc0")
                with tc.tile_pool(name="ps_gn", bufs=2, space="PSUM") as ps_gn:
                    eps_sb = small.tile([GPT, 1], F32, tag="eps_c", bufs=1)
                    nc.gpsimd.memset(eps_sb, float(EPS))
                    sq_scratch = small.tile([128, HW], BF16, tag="sqs",
                                            bufs=1)
                    sts = []

                    cp_scratch = small.tile([128, HW], BF16, tag="cps",
                                            bufs=1)

                    def gn_stats(t):
                        st = small.tile([128, 2], F32, tag=f"st{t}", bufs=1,
                                        name=f"st{t}")
                        sts.append(st)
                        nc.scalar.activation(
                            out=cp_scratch, in_=x_sb[t][:, :],
                            func=mybir.ActivationFunctionType.Copy,
                            accum_out=st[:, 0:1])
                        nc.vector.scalar_tensor_tensor(
                            out=sq_scratch, in0=x_sb[t][:, :], scalar=1.0,
                            in1=x_sb[t][:, :],
                            op0=mybir.AluOpType.bypass, op1=MULT,
                            accum_out=st[:, 1:2])

                    def gn_tile(t):
                        st = sts[t]
                        gpsum = ps_gn.tile([GPT, 2], F32, tag="gps")
                        nc.tensor.matmul(gpsum[:, :], lhsT=sel_sb[:, :],
                                         rhs=st[:, :], start=True, stop=True)
                        # grp cols: 0 rstd, 1 mean*rstd, 2 mean, 3 E[x^2]
                        grp = small.tile([GPT, 4], F32, tag="grp", bufs=2,
                                         name=f"grp{t}")
                        nc.vector.tensor_scalar_mul(grp[:, 2:4],
                                                    gpsum[:, 0:2], GN_INV)
                        nc.vector.tensor_mul(grp[:, 0:1], grp[:, 2:3],
                                             grp[:, 2:3])
                        nc.vector.tensor_sub(grp[:, 0:1], grp[:, 3:4],
                                             grp[:, 0:1])
                        nc.scalar.activation(
                            out=grp[:, 0:1], in_=grp[:, 0:1],
                            func=mybir.ActivationFunctionType.Sqrt,
                            bias=eps_sb[:, :], scale=1.0)
                        nc.vector.reciprocal(out=grp[:, 0:1], in_=grp[:, 0:1])
                        nc.vector.tensor_mul(grp[:, 1:2], grp[:, 2:3],
                                             grp[:, 0:1])
                        epsum = ps_gn.tile([128, 2], F32, tag="eps")
                        nc.tensor.matmul(epsum[:, :], lhsT=selT_sb[:, :],
                                         rhs=grp[:, 0:2], start=True,
                                         stop=True)
                        ga = small.tile([128, 1], F32, tag=f"ga{t}", bufs=1,
                                        name=f"ga{t}")
                        gd = small.tile([128, 1], F32, tag=f"gd{t}", bufs=1,
                                        name=f"gd{t}")
                        nc.vector.tensor_mul(ga[:, :], gamma_sb[:, t:t + 1],
                                             epsum[:, 0:1])
                        nc.vector.tensor_mul(gd[:, :], gamma_sb[:, t:t + 1],
                                             epsum[:, 1:2])
                        nc.vector.tensor_sub(gd[:, :], beta_sb[:, t:t + 1],
                                             gd[:, :])
                        nc.vector.tensor_scalar(
                            out=h_pair[t // 2][:, t % 2, :],
                            in0=x_sb[t][:, :],
                            scalar1=ga[:, :], scalar2=gd[:, :],
                            op0=MULT, op1=ADD)

                    # stats(t) immediately before tile t's chain: the
                    # gpsum matmul then fires as soon as its own stats land
                    for t in range(CT):
                        gn_stats(t)
                        gn_tile(t)
                        if t % 2 == 1:
                            a = t // 2
                            for n in range(2):
                                nc.tensor.matmul(
                                    ppq0[:, 512 * n:512 * (n + 1)],
                                    lhsT=wqk_sb[a][:, :, 0:128],
                                    rhs=h_pair[a][:, :, 512 * n:512 * (n + 1)],
                                    start=(a == 0), stop=(a == 1),
                                    perf_mode=DR)
                                nc.tensor.matmul(
                                    ppk0[:, 512 * n:512 * (n + 1)],
                                    lhsT=wqk_sb[a][:, :, HID:HID + 128],
                                    rhs=h_pair[a][:, :, 512 * n:512 * (n + 1)],
                                    start=(a == 0), stop=(a == 1),
                                    perf_mode=DR)
                nc.vector.tensor_scalar(
                    out=q_sb[0], in0=ppq0[:, :],
                    scalar1=b_in_sb[:, 0:1], scalar2=None, op0=ADD)
                # k0 evict on ACT (idle pre-exp) so it runs parallel to the
                # q0 evict on DVE — both gate the first logits matmul.
                nc.scalar.activation(
                    out=k_sb[0], in_=ppk0[:, :],
                    func=mybir.ActivationFunctionType.Identity,
                    bias=b_in_sb[:, 4:5], scale=1.0)
                for a in range(NA):
                    for i in range(2):
                        nc.vector.memset(vT_pair[a][:, i, :, HD:HD + 1], 1.0)
                ind2_sb = const.tile([2, 128], F32)
                nc.sync.dma_start(out=ind2_sb,
                                  in_=selT_ext[GPT:GPT + 2, :])
                with tc.tile_pool(name="pv", bufs=2, space="PSUM") as pv:
                    emit_logits_exp(0, fillers={
                        0: [lambda: v_one(0)],
                        1: [lambda: v_one(1)],
                        2: [lambda: qk_burst(1, "q")],
                        3: [lambda: v_one(2)],
                        4: [lambda: v_one(3)],
                        5: [lambda: qk_burst(1, "k")],
                        6: [lambda: v_one(4), lambda: v_one(5)],
                        7: [lambda: v_one(6), lambda: v_one(7)],
                    })
                with tc.tile_pool(name="po", bufs=2, space="PSUM") as pop:
                    emit_logits_exp(1, out2_of=0, o2_defer=(1,), fillers={
                        2: [lambda: qk_burst(2, "q")],
                        5: [lambda: qk_burst(2, "k")],
                    })
                    finish_den(0)
                    emit_logits_exp(2, out2_of=1, o2_defer=(1,), fillers={
                        2: [lambda: qk_burst(3, "q")],
                        5: [lambda: qk_burst(3, "k")],
                    })
                    finish_den(1)
                    finish_mul(0)
                    emit_logits_exp(3, out2_of=2)
                    finish_den(2)
                    finish_mul(1)
                    # chase pair 3 sub-major: sub0's po finishes (and its
                    # au cast + den extract start) while sub1 still matmuls
                    for a in range(NA):
                        out2_step(3, a, subs=(0,))
                    for a in range(NA):
                        out2_step(3, a, subs=(1,))
                    finish_den(3, pe_bcast=True)
                    finish_mul(2)
                    finish_mul(3)

            # ---------- proj_out + bias + residual ----------
            # DoubleRow over chan-tile pairs: 2 accumulation steps per chunk.
            # pre = step a=0 (runs as soon as attn_pair[0] lands), fin = a=1
            # + evict + bf16 DMA out.
            with tc.tile_pool(name="ps_pout", bufs=8, space="PSUM") as ps_pout:
                # ring of 8: every chunk's first DoubleRow accumulation can
                # run during the attention tail; after mul(3) only the 8
                # closing matmuls + evicts remain
                def pout_chunk_pre(m, n, pps):
                    pp = ps_pout.tile([128, 512], F32, tag="pp",
                                      name=f"po_{m}_{n}")
                    pps[(m, n)] = pp
                    nc.tensor.matmul(
                        pp[:, :],
                        lhsT=wo_sb[0][:, :, 128 * m:128 * (m + 1)],
                        rhs=attn_pair[0][:, :, 512 * n:512 * (n + 1)],
                        start=True, stop=False, perf_mode=DR)

                def pout_chunk_fin(m, n, pps):
                    pp = pps.pop((m, n))
                    nc.tensor.matmul(
                        pp[:, :],
                        lhsT=wo_sb[1][:, :, 128 * m:128 * (m + 1)],
                        rhs=attn_pair[1][:, :, 512 * n:512 * (n + 1)],
                        start=False, stop=True, perf_mode=DR)
                    o_sb = small.tile([128, 512], BF16, tag="osb", bufs=4)
                    nc.vector.scalar_tensor_tensor(
                        out=o_sb, in0=pp[:, :],
                        scalar=b_out_sb[:, m:m + 1],
                        in1=x_sb[m][:, 512 * n:512 * (n + 1)],
                        op0=ADD, op1=ADD)
                    for hh in range(2):
                        deng = nc.gpsimd if (2 * m + n + hh) % 2 else nc.sync
                        deng.dma_start(
                            out=out_ext[128 * m:128 * (m + 1),
                                        512 * n + 256 * hh:
                                        512 * n + 256 * (hh + 1)],
                            in_=o_sb[:, 256 * hh:256 * (hh + 1)])

                pps = {}
                chunks = [(m, n) for n in range(2) for m in range(CT)]
                for m, n in chunks:
                    pout_chunk_pre(m, n, pps)
                for m, n in chunks:
                    pout_chunk_fin(m, n, pps)
    return nc


def _install_ntff_hook():
    """The agent image's antenv lacks axon_hooks; synthesize it so
    run_bass_kernel_spmd(trace=True) can reach the NTFF profiler."""
    import types
    if "antenv.axon_hooks" in sys.modules:
        return
    mod = types.ModuleType("antenv.axon_hooks")
    mod._hook = None

    def set_axon_ntff_profile_hook(hook):
        mod._hook = hook

    def get_axon_ntff_profile_hook():
        return mod._hook

    mod.set_axon_ntff_profile_hook = set_axon_ntff_profile_hook
    mod.get_axon_ntff_profile_hook = get_axon_ntff_profile_hook
    sys.modules["antenv.axon_hooks"] = mod
    try:
        from trn_agent_boot.trn_boot import _ntff_profile_via_ctypes
        hook = _ntff_profile_via_ctypes("/opt/axon/libaxon_pjrt.so")
        if hook is not None:
            set_axon_ntff_profile_hook(hook)
    except Exception as e:  # degrade to no tracing
        print("ntff hook setup failed:", e)


_COMPILED = None


def _get_compiled():
    global _COMPILED
    if _COMPILED is None:
        nc = build_graph()
        nc.compile()
        _COMPILED = nc
    return _COMPILED


def _make_consts():
    # within any 128-channel tile, partition p belongs to local group p//16;
    # rows GPT/GPT+1 are even/odd-head indicator rows for the den broadcast
    sel = np.zeros((128, GPT), dtype=np.float32)
    selT = np.zeros((GPT + 2, 128), dtype=np.float32)
    for p in range(128):
        sel[p, p // GS] = 1.0
        selT[p // GS, p] = 1.0
    selT[GPT, 0:64] = 1.0
    selT[GPT + 1, 64:128] = 1.0
    return sel, selT


def _pm(v, cols):
    """[cols*128] vector -> partition-major [128, cols]."""
    return np.ascontiguousarray(v.reshape(cols, 128).T)


def _pack_pairs(w, m_dim):
    """w [m_dim, 512] -> [128, 2*2*m_dim] fp8: flat[a] [p, i, m] =
    w[m, 128*(2a+i)+p]."""
    wT = w.T.reshape(2, 2, 128, m_dim)           # [a, i, p, m]
    out = np.transpose(wT, (0, 2, 1, 3))          # [a, p, i, m]
    out = out.reshape(2, 128, 2 * m_dim)          # [a, p, i*m]
    out = np.concatenate([out[0], out[1]], axis=1)  # [p, a*i*m]
    return np.ascontiguousarray(out).astype(ml_dtypes.float8_e4m3)


def kernel(x, gamma, beta, w_in, b_in, w_out, b_out, _trace=False):
    x = np.asarray(x, dtype=np.float32)
    gamma = np.asarray(gamma, dtype=np.float32)
    beta = np.asarray(beta, dtype=np.float32)
    w_in = np.asarray(w_in, dtype=np.float32)
    b_in = np.asarray(b_in, dtype=np.float32)
    w_out = np.asarray(w_out, dtype=np.float32)
    b_out = np.asarray(b_out, dtype=np.float32)

    wqk8 = _pack_pairs(w_in[0:2 * HID], 2 * HID)   # q rows 0:512, k 512:1024
    wv8 = _pack_pairs(w_in[2 * HID:3 * HID], HID)
    wo8 = _pack_pairs(w_out, HID)
    sel, selT = _make_consts()
    # fold v-bias through proj_out: softmax rows sum to 1, so the attention
    # output is attn_raw + b_v exactly; w_out @ b_v + b_out replaces b_out.
    b_v = b_in[2 * HID:3 * HID]
    b_out_eff = b_out + w_out.astype(np.float64) @ b_v.astype(np.float64)
    b_out_eff = b_out_eff.astype(np.float32)
    cpack = np.zeros((128, 28), dtype=np.float32)
    cpack[:, 0:4] = _pm(gamma, CT)
    cpack[:, 4:8] = _pm(beta, CT)
    cpack[:, 8:16] = _pm(b_in[0:2 * HID], 8)
    cpack[:, 16:20] = _pm(b_out_eff, CT)
    cpack[:, 20:28] = sel
    common = {
        "wqk8": wqk8,
        "wv8": wv8,
        "wo8": wo8,
        "cpack": cpack,
        "gn_selT": selT,
    }
    in_maps = []
    for b in range(B):
        m = dict(common)
        m["x"] = np.ascontiguousarray(x[b].reshape(C, HW)).astype(
            ml_dtypes.bfloat16)
        in_maps.append(m)

    if _trace:
        _install_ntff_hook()
    nc = _get_compiled()
    res = run_bass_kernel_spmd(nc, in_maps, core_ids=list(range(B)),
                               trace=_trace)
    out = np.stack([np.asarray(res.results[b]["out"]).astype(np.float32)
                    .reshape(C, H, W) for b in range(B)])
    if _trace:
        return out, res
    return out


if __name__ == "__main__":
    rng = np.random.default_rng(0)
    inputs = {
        "x": rng.standard_normal((B, C, H, W), dtype=np.float32),
        "gamma": np.ones(C, dtype=np.float32),
        "beta": np.zeros(C, dtype=np.float32),
        "w_in": (rng.standard_normal((3 * HID, C), dtype=np.float32)
                 / np.sqrt(C)),
        "b_in": np.zeros(3 * HID, dtype=np.float32),
        "w_out": (rng.standard_normal((C, HID), dtype=np.float32)
                  / np.sqrt(HID)),
        "b_out": np.zeros(C, dtype=np.float32),
    }
    out = kernel(**inputs)
    print("kernel ran, out shape", out.shape)
